# revision 1
# baseline (speedup 1.0000x reference)
"""NemotronH Mamba2 mixer on 8 Trainium2 cores (Bass/Tile).

Sharding: tensor-parallel over heads/groups. Core c owns group c =
16 heads (= 1024 gate/x channels, 128 B + 128 C state channels, 16 dt).
in_proj rows and out_proj columns are sharded accordingly; out_proj is
row-parallel over the contraction, partials are combined on the host.

Per-core dataflow (seq superblocks of 512, SSD chunks of 128):
  in_proj (fp32r matmul, weights pre-tiled for single-DMA loads)
  -> depthwise conv taps on DVE + SiLU
  -> Mamba2 chunked SSD: per-head decay matrices built with a PE
     broadcast matmul (indicator x cs), Ydiag + Yoff accumulated in one
     PSUM group per head
  -> gated group RMSNorm -> out_proj (fp32r) -> partial [4096, 2048].
"""

import numpy as np

import concourse.bass as bass
import concourse.mybir as mybir
from concourse import bacc
from concourse.tile import TileContext
from concourse.bass_utils import run_bass_kernel_spmd

F32 = mybir.dt.float32
F32R = mybir.dt.float32r
AF = mybir.ActivationFunctionType
ALU = mybir.AluOpType

# Model dims
H_SIZE = 4096
NH = 128
HD = 64
SS = 128
KCONV = 4
NG = 8
CHUNK = 128
INTER = NH * HD                 # 8192
CONV_DIM = INTER + 2 * NG * SS  # 10240
PROJ = INTER + CONV_DIM + NH    # 18560
DT_MIN, DT_MAX = 0.001, 100.0
EPS = 1e-5
GROUP = INTER // NG             # 1024

# Sharding / tiling
N_CORES = 8
S = 2048
HL = NH // N_CORES              # 16 local heads
CLOC = HL * HD                  # 1024 local gate/x channels
NSB = 4                         # seq superblocks
SB = S // NSB                   # 512
NCPB = SB // CHUNK              # 4 chunks per superblock
NCH = S // CHUNK                # 16 chunks
NF = 19                         # in_proj f-tiles (2432 = 19*128, padded)
NK1 = H_SIZE // 128             # 32 k-tiles for in_proj
NK2 = CLOC // 128               # 8 k-tiles for out_proj
NM2 = H_SIZE // 128             # 32 m-tiles for out_proj
NEGM = -1e30

# log1p(u)/u on [0,1], Chebyshev-fit degree 12 (max fp32 err ~1.1e-7)
LOG1P_C = [0.9999999999815061, -0.4999999935552795, 0.33333295899388315,
           -0.2499913901062215, 0.19989602251462296, -0.1659083573590588,
           0.1392317246686566, -0.1130135727826319, 0.08261769871302305,
           -0.04960969557400616, 0.021956439674455992, -0.006180556818034449,
           0.0008159022224092772]

_CACHE = {}


def r32(ap):
    return ap.bitcast(F32R)


def build_nc():
    nc = bacc.Bacc(None, target_bir_lowering=False)

    # hidden, pre-tiled: [sb, half, 128, 16*512] (per-partition contiguous)
    hids = nc.declare_dram_parameter("hids", [NSB, 2, 128, 16 * SB], F32,
                                     isOutput=False)
    # in_proj weights, pre-tiled per f-tile: [f, half, 128, 16*128]
    w1f = nc.declare_dram_parameter("w1f", [NF, 2, 128, 16 * 128], F32,
                                    isOutput=False)
    # out_proj weights, pre-tiled per m-tile: [m, 128, 8*128]
    w2m = nc.declare_dram_parameter("w2m", [NM2, 128, NK2 * 128], F32,
                                    isOutput=False)
    convw = nc.declare_dram_parameter("convw", [128, 10 * KCONV], F32,
                                      isOutput=False)
    convb = nc.declare_dram_parameter("convb", [128, 10], F32, isOutput=False)
    dtbias = nc.declare_dram_parameter("dtbias", [HL, 1], F32, isOutput=False)
    acol = nc.declare_dram_parameter("acol", [HL, 1], F32, isOutput=False)
    dbc = nc.declare_dram_parameter("dbc", [128, HL], F32, isOutput=False)
    negmask = nc.declare_dram_parameter("negmask", [128, 128], F32,
                                        isOutput=False)
    ident = nc.declare_dram_parameter("ident", [128, 128], F32, isOutput=False)
    e127 = nc.declare_dram_parameter("e127", [128, 1], F32, isOutput=False)
    outp = nc.declare_dram_parameter("outp", [NM2, 4, 128, 512], F32,
                                     isOutput=True)

    with TileContext(nc) as tc:
        with tc.tile_pool(name="const", bufs=1) as cp, \
             tc.tile_pool(name="dram", bufs=1, space="DRAM") as dp:
            id_sb = cp.tile([128, 128], F32, tag="id")
            nm_sb = cp.tile([128, 128], F32, tag="nm")
            dbc_sb = cp.tile([128, HL], F32, tag="dbc")
            cw_sb = cp.tile([128, 10 * KCONV], F32, tag="cw")
            cb_sb = cp.tile([128, 10], F32, tag="cb")
            dtb_sb = cp.tile([HL, 1], F32, tag="dtb")
            a_sb = cp.tile([HL, 1], F32, tag="acol")
            e127_sb = cp.tile([128, 1], F32, tag="e127")
            ones16 = cp.tile([HL, CHUNK], F32, tag="ones16")
            st_sb = cp.tile([128, HL * HD], F32, tag="state")
            nc.sync.dma_start(out=id_sb[:], in_=ident[:])
            nc.sync.dma_start(out=nm_sb[:], in_=negmask[:])
            nc.sync.dma_start(out=dbc_sb[:], in_=dbc[:])
            nc.sync.dma_start(out=cw_sb[:], in_=convw[:])
            nc.sync.dma_start(out=cb_sb[:], in_=convb[:])
            nc.sync.dma_start(out=dtb_sb[:], in_=dtbias[:])
            nc.sync.dma_start(out=a_sb[:], in_=acol[:])
            nc.sync.dma_start(out=e127_sb[:], in_=e127[:])
            nc.vector.memset(ones16[:], 1.0)
            nc.vector.memset(st_sb[:], 0.0)

            _main_phase(nc, tc, hids, w1f, id_sb, nm_sb, dbc_sb,
                        cw_sb, cb_sb, dtb_sb, a_sb, e127_sb, ones16,
                        st_sb, w2m, outp)

    nc.compile()
    return nc


def _main_phase(nc, tc, hids, w1f, id_sb, nm_sb, dbc_sb,
                cw_sb, cb_sb, dtb_sb, a_sb, e127_sb, ones16,
                st_sb, w2m, outp):
    with tc.tile_pool(name="hid", bufs=2) as hidp, \
         tc.tile_pool(name="w1", bufs=3) as w1p, \
         tc.tile_pool(name="gq", bufs=2) as gqp, \
         tc.tile_pool(name="conv", bufs=1) as convp, \
         tc.tile_pool(name="dtl", bufs=1) as dtp, \
         tc.tile_pool(name="dtr", bufs=2) as dtrp, \
         tc.tile_pool(name="cch", bufs=2) as cchp, \
         tc.tile_pool(name="chunk", bufs=2) as chp, \
         tc.tile_pool(name="chunk1", bufs=1) as ch1p, \
         tc.tile_pool(name="w2", bufs=2) as w2p, \
         tc.tile_pool(name="oev", bufs=2) as oevp, \
         tc.tile_pool(name="heads", bufs=1) as hp, \
         tc.tile_pool(name="psA", bufs=1, space="PSUM") as psA, \
         tc.tile_pool(name="psS", bufs=1, space="PSUM") as psS, \
         tc.tile_pool(name="psT", bufs=1, space="PSUM") as psT, \
         tc.tile_pool(name="psY", bufs=2, space="PSUM") as psY:

        # convcat: 10 conv channel tiles (8 x, 1 B, 1 C), each 3 halo + 512
        ccat = convp.tile([128, 10 * (SB + 3)], F32, tag="ccat")
        for t in range(10):
            nc.vector.memset(ccat[:, t * (SB + 3):t * (SB + 3) + 3], 0.0)

        pending_out = []

        def emit_outproj(m, qst, sbq):
            w2 = w2p.tile([128, NK2 * 128], F32R, tag="w2")
            nc.sync.dma_start(out=w2[:], in_=r32(w2m[m]))
            acc = psA.tile([128, 512], F32, tag="ipacc")
            for kt in range(NK2):
                nc.tensor.matmul(
                    acc[:], w2[:, kt * 128:(kt + 1) * 128],
                    qst[:, kt * SB:kt * SB + SB],
                    start=(kt == 0), stop=(kt == NK2 - 1))
            ev = oevp.tile([128, 512], F32, tag="oev")
            nc.scalar.copy(ev[:], acc[:])
            nc.sync.dma_start(out=outp[m, sbq], in_=ev[:])

        for sb in range(NSB):
            # ---------------- in_proj for this superblock ----------------
            halves = []
            for khalf in range(2):
                hid_h = hidp.tile([128, 16 * SB], F32R, tag="hid")
                nc.sync.dma_start(out=hid_h[:], in_=r32(hids[sb, khalf]))
                halves.append(hid_h)

            gate_sb = gqp.tile([128, 8 * SB], F32, tag="gq")
            dtraw = dtrp.tile([HL, SB], F32, tag="dtraw")

            # halo copies must read previous superblock before overwrite
            if sb > 0:
                for t in range(10):
                    base = t * (SB + 3)
                    nc.vector.tensor_copy(
                        ccat[:, base:base + 3], ccat[:, base + SB:base + SB + 3])

            def emit_ftile(f, gate_sb=gate_sb, dtraw=dtraw, halves=halves):
                w1h = []
                for khalf in range(2):
                    w1t_ = w1p.tile([128, 16 * 128], F32R, tag="w1")
                    nc.sync.dma_start(out=w1t_[:], in_=r32(w1f[f, khalf]))
                    w1h.append(w1t_)
                acc = psA.tile([128, SB], F32, tag="ipacc")
                for k in range(NK1):
                    nc.tensor.matmul(
                        acc[:],
                        w1h[k // 16][:, (k % 16) * 128:(k % 16 + 1) * 128],
                        halves[k // 16][:, (k % 16) * SB:(k % 16 + 1) * SB],
                        start=(k == 0), stop=(k == NK1 - 1))
                if f < 8:
                    nc.scalar.copy(gate_sb[:, f * SB:(f + 1) * SB], acc[:])
                elif f < 18:
                    t = f - 8
                    base = t * (SB + 3)
                    nc.scalar.copy(ccat[:, base + 3:base + 3 + SB], acc[:])
                else:
                    nc.scalar.copy(dtraw[:, :], acc[:HL, :])

            for f in [18] + list(range(18)):
                emit_ftile(f)
                for _ in range(2):
                    if pending_out:
                        emit_outproj(*pending_out.pop(0))

            nc.scalar.activation(gate_sb[:], gate_sb[:], AF.Silu)

            # ---------------- dt pipeline ----------------
            # softplus(z) = relu(z) + log1p(exp(-|z|)); log1p via poly
            # (no Softplus/Ln activation table on gen3)
            uu = dtp.tile([HL, SB], F32, tag="uu")
            pp = dtrp.tile([HL, SB], F32, tag="pp")
            dtsp = dtraw  # in-place: relu(z) overwrites z
            cs = pp       # reuse pp once the poly is folded in
            nc.scalar.activation(dtraw[:], dtraw[:], AF.Identity,
                                 bias=dtb_sb[:, 0:1])
            # uu = exp(min(z, -z)) = exp(-|z|)
            nc.vector.tensor_scalar(uu[:], dtraw[:], -1.0, None, ALU.mult)
            nc.vector.tensor_tensor(uu[:], uu[:], dtraw[:], ALU.min)
            nc.scalar.activation(uu[:], uu[:], AF.Exp)
            # Horner for q(u) = log1p(u)/u
            nc.vector.tensor_scalar(pp[:], uu[:], LOG1P_C[-1], LOG1P_C[-2],
                                    ALU.mult, ALU.add)
            for cidx in range(len(LOG1P_C) - 3, -1, -1):
                nc.vector.tensor_tensor(pp[:], pp[:], uu[:], ALU.mult)
                nc.vector.tensor_scalar(pp[:], pp[:], LOG1P_C[cidx], None,
                                        ALU.add)
            nc.vector.tensor_tensor(pp[:], pp[:], uu[:], ALU.mult)
            relu_t = uu  # uu dead; use as relu scratch
            nc.scalar.activation(relu_t[:], dtraw[:], AF.Relu)
            nc.vector.tensor_tensor(dtsp[:], relu_t[:], pp[:], ALU.add)
            nc.vector.tensor_scalar(dtsp[:], dtsp[:], DT_MIN, DT_MAX,
                                    ALU.max, ALU.min)
            dA = uu  # reuse again (relu scratch is dead)
            nc.vector.tensor_scalar(dA[:], dtsp[:], a_sb[:, 0:1], None,
                                    ALU.mult)
            for cl in range(NCPB):
                nc.vector.tensor_tensor_scan(
                    cs[:, cl * CHUNK:(cl + 1) * CHUNK],
                    ones16[:], dA[:, cl * CHUNK:(cl + 1) * CHUNK],
                    0.0, ALU.mult, ALU.add)

            # ---------------- SSD chunks ----------------
            qstage = gqp.tile([128, NK2 * SB], F32R, tag="gq")

            def emit_conv(cl):
                xc = cchp.tile([128, 8 * CHUNK], F32, tag="xc")
                bcs = ch1p.tile([128, CHUNK], F32, tag="bc")
                ccs = ch1p.tile([128, CHUNK], F32, tag="cc")
                for t in range(10):
                    base = t * (SB + 3) + cl * CHUNK
                    dst = (xc[:, t * CHUNK:(t + 1) * CHUNK] if t < 8
                           else (bcs[:] if t == 8 else ccs[:]))
                    nc.vector.tensor_scalar(
                        dst, ccat[:, base:base + CHUNK],
                        cw_sb[:, t * KCONV:t * KCONV + 1], cb_sb[:, t:t + 1],
                        ALU.mult, ALU.add)
                    for j in range(1, KCONV):
                        nc.vector.scalar_tensor_tensor(
                            dst, ccat[:, base + j:base + j + CHUNK],
                            cw_sb[:, t * KCONV + j:t * KCONV + j + 1], dst,
                            ALU.mult, ALU.add)
                    nc.scalar.activation(dst, dst, AF.Silu)
                return xc, bcs, ccs

            for cl in range(NCPB):
                ch = sb * NCPB + cl
                csl = slice(cl * CHUNK, (cl + 1) * CHUNK)
                xc, bcs, ccs = emit_conv(cl)

                # gate transpose + SiLU
                gps = psT.tile([128, CLOC], F32, tag="trans")
                for t in range(8):
                    nc.tensor.transpose(
                        gps[:, t * 128:(t + 1) * 128],
                        gate_sb[:, t * SB + cl * CHUNK:t * SB + (cl + 1) * CHUNK],
                        id_sb[:])
                silg = ch1p.tile([128, CLOC], F32, tag="silg")
                nc.scalar.copy(silg[:], gps[:])

                # small transposes: csT, dtT
                pT = psS.tile([128, 128], F32, tag="small")
                nc.tensor.transpose(pT[:, :HL], cs[:, csl], id_sb[:HL, :HL])
                csT = chp.tile([128, HL], F32, tag="csT")
                negcsT = chp.tile([128, HL], F32, tag="negcsT")
                nc.scalar.copy(csT[:], pT[:, :HL])
                nc.scalar.mul(negcsT[:], pT[:, :HL], -1.0)

                pT2 = psS.tile([128, 128], F32, tag="small")
                nc.tensor.transpose(pT2[:, :HL], dtsp[:, csl], id_sb[:HL, :HL])
                dtT = chp.tile([128, HL], F32, tag="dtT")
                nc.scalar.copy(dtT[:], pT2[:, :HL])

                # cs at chunk end, broadcast across partitions (PE matmul)
                pT3 = psS.tile([128, 128], F32, tag="small")
                e127b = bass.AP(tensor=e127_sb.tensor,
                                offset=e127_sb[:].offset,
                                ap=[[e127_sb[:].ap[0][0], 128], [0, 128]])
                nc.tensor.matmul(pT3[:, :HL], e127b, csT[:],
                                 start=True, stop=True)
                cdbc = chp.tile([128, HL], F32, tag="cdbc")
                decT = chp.tile([128, HL], F32, tag="decT")
                nc.scalar.activation(cdbc[:], pT3[:, :HL], AF.Exp)
                nc.vector.tensor_tensor(decT[:], pT3[:, :HL], csT[:],
                                        ALU.subtract)
                nc.scalar.activation(decT[:], decT[:], AF.Exp)
                ddt = chp.tile([128, HL], F32, tag="ddt")
                nc.vector.tensor_tensor(ddt[:], dtT[:], decT[:], ALU.mult)

                # x transpose -> xT, then xdt / xdd
                xps = psT.tile([128, CLOC], F32, tag="trans")
                for t in range(8):
                    nc.tensor.transpose(
                        xps[:, t * 128:(t + 1) * 128],
                        xc[:, t * CHUNK:(t + 1) * CHUNK], id_sb[:])
                xT = ch1p.tile([128, CLOC], F32, tag="xT")
                nc.scalar.copy(xT[:], xps[:])
                xdt = ch1p.tile([128, CLOC], F32, tag="xdt")
                xdd = ch1p.tile([128, CLOC], F32R, tag="xdd")
                for h in range(HL):
                    hs = slice(h * HD, (h + 1) * HD)
                    nc.vector.tensor_scalar(
                        xdt[:, hs], xT[:, hs], dtT[:, h:h + 1], None, ALU.mult)
                for h in range(HL):
                    hs = slice(h * HD, (h + 1) * HD)
                    nc.vector.tensor_scalar(
                        xdd[:, hs], xT[:, hs], ddt[:, h:h + 1], None, ALU.mult)

                # B chunk transposed (B_LN)
                pbt = psS.tile([128, 128], F32, tag="small")
                nc.tensor.transpose(pbt[:], bcs[:], id_sb[:])
                bln = chp.tile([128, 128], F32R, tag="bln")
                nc.scalar.copy(bln[:], pbt[:])

                # Gram^T = B C^T in [s, l]; evicted to SBUF
                gram_ps = psS.tile([128, 128], F32, tag="small")
                nc.tensor.matmul(gram_ps[:], bcs[:], ccs[:],
                                 start=True, stop=True)
                gram = ch1p.tile([128, 128], F32, tag="gram")
                nc.scalar.copy(gram[:], gram_ps[:])

                # per-head decay matrices in groups of 4 heads
                y_ps = psY.tile([128, CLOC], F32, tag="yo")
                for g in range(HL // 4):
                    pb4 = psS.tile([128, 512], F32, tag="small")
                    for j in range(4):
                        h = 4 * g + j
                        idcol = id_sb[:HL, h:h + 1]
                        indh = bass.AP(tensor=idcol.tensor,
                                       offset=idcol.offset,
                                       ap=[[idcol.ap[0][0], HL], [0, 128]])
                        nc.tensor.matmul(pb4[:, j * 128:(j + 1) * 128], indh,
                                         cs[:, csl], start=True, stop=True)
                    epb4 = hp.tile([128, 512], F32, tag="epb")
                    nc.scalar.activation(epb4[:], pb4[:], AF.Exp)
                    seg4 = hp.tile([128, 512], F32, tag="seg")
                    for j in range(4):
                        h = 4 * g + j
                        nc.vector.scalar_tensor_tensor(
                            seg4[:, j * 128:(j + 1) * 128],
                            pb4[:, j * 128:(j + 1) * 128],
                            negcsT[:, h:h + 1], nm_sb[:], ALU.add, ALU.add)
                    nc.scalar.activation(seg4[:], seg4[:], AF.Exp)
                    gram_b = bass.AP(tensor=gram.tensor, offset=gram[:].offset,
                                     ap=[gram[:].ap[0], [0, 4], [1, 128]])
                    ccs_b = bass.AP(tensor=ccs.tensor, offset=ccs[:].offset,
                                    ap=[ccs[:].ap[0], [0, 4], [1, 128]])
                    s4 = seg4[:].rearrange("p (j l) -> p j l", j=4)
                    e4 = epb4[:].rearrange("p (j l) -> p j l", j=4)
                    nc.vector.tensor_tensor(s4, s4, gram_b, ALU.mult)
                    nc.vector.tensor_tensor(e4, e4, ccs_b, ALU.mult)
                    for j in range(4):
                        h = 4 * g + j
                        hs = slice(h * HD, (h + 1) * HD)
                        nc.tensor.matmul(
                            y_ps[:, hs], seg4[:, j * 128:(j + 1) * 128],
                            xdt[:, hs], start=True, stop=False)
                        nc.tensor.matmul(
                            y_ps[:, hs], epb4[:, j * 128:(j + 1) * 128],
                            st_sb[:, hs], start=False, stop=True)

                # states for this chunk
                s_ps = psY.tile([128, CLOC], F32, tag="yo")
                for half in range(2):
                    hsl = slice(half * 512, (half + 1) * 512)
                    nc.tensor.matmul(
                        s_ps[:, hsl], bln[:], xdd[:, hsl],
                        start=True, stop=True)

                # y = (Ydiag + Yoff) + D*x ; state update
                y_sb = ch1p.tile([128, CLOC], F32, tag="ysb")
                for h in range(HL):
                    hs = slice(h * HD, (h + 1) * HD)
                    nc.vector.scalar_tensor_tensor(
                        y_sb[:, hs], xT[:, hs], dbc_sb[:, h:h + 1],
                        y_ps[:, hs], ALU.mult, ALU.add)
                for h in range(HL):
                    hs = slice(h * HD, (h + 1) * HD)
                    nc.vector.scalar_tensor_tensor(
                        st_sb[:, hs], st_sb[:, hs], cdbc[:, h:h + 1],
                        s_ps[:, hs], ALU.mult, ALU.add)

                # gate + group RMSNorm
                nc.vector.tensor_tensor(y_sb[:], y_sb[:], silg[:], ALU.mult)
                ssum = ch1p.tile([128, 1], F32, tag="ssum")
                # Square's main output is discarded into xdd (scratch)
                nc.scalar.activation(xdd[:], y_sb[:], AF.Square,
                                     accum_out=ssum[:, 0:1])
                nc.vector.tensor_scalar(ssum[:], ssum[:], 1.0 / GROUP, EPS,
                                        ALU.mult, ALU.add)
                rstd = chp.tile([128, 1], F32, tag="rstd")
                tnew = chp.tile([128, 1], F32, tag="tnew")
                nc.scalar.activation(tnew[:], ssum[:], AF.Sqrt)
                nc.vector.reciprocal(rstd[:], tnew[:])
                normed = ch1p.tile([128, CLOC], F32, tag="normed")
                nc.vector.tensor_scalar(
                    normed[:], y_sb[:], rstd[:, 0:1], None, ALU.mult)

                # transpose normed -> [c, s] and stage out to DRAM
                nps = psT.tile([128, CLOC], F32, tag="trans")
                for t in range(8):
                    nc.tensor.transpose(
                        nps[:, t * 128:(t + 1) * 128],
                        normed[:, t * 128:(t + 1) * 128], id_sb[:])
                qdst = qstage[:].rearrange(
                    "p (t s) -> p t s", t=NK2)[:, :, cl * 128:(cl + 1) * 128]
                nsrc = nps[:].rearrange("p (t s) -> p t s", t=NK2)
                nc.scalar.copy(qdst, nsrc)

            # out_proj m-blocks are deferred and interleaved into the
            # next superblock's in_proj f-loop (shared psA rotation)
            pending_out.extend((m, qstage, sb) for m in range(NM2))

        while pending_out:
            emit_outproj(*pending_out.pop(0))


def prepare_in_maps(hidden_states, in_proj_w, conv_w, conv_b, dt_bias, D,
                    norm_w, out_proj_w):
    hidT = np.ascontiguousarray(hidden_states.reshape(S, H_SIZE).T)
    # [half, kk, r, sb, c] -> [sb, half, r, kk, c]
    hids = np.ascontiguousarray(
        hidT.reshape(2, 16, 128, NSB, SB).transpose(3, 0, 2, 1, 4)
        .reshape(NSB, 2, 128, 16 * SB))
    negmask = np.where(np.arange(128)[None, :] >= np.arange(128)[:, None],
                       np.float32(0.0), np.float32(NEGM)).astype(np.float32)
    ident = np.eye(128, dtype=np.float32)
    e127 = np.zeros((128, 1), np.float32)
    e127[127, 0] = 1.0
    in_maps = []
    for c in range(N_CORES):
        gsl = slice(CLOC * c, CLOC * (c + 1))
        xsl = slice(INTER + CLOC * c, INTER + CLOC * (c + 1))
        bsl = slice(2 * INTER + SS * c, 2 * INTER + SS * (c + 1))
        cslc = slice(2 * INTER + NG * SS + SS * c,
                     2 * INTER + NG * SS + SS * (c + 1))
        dsl = slice(INTER + CONV_DIM + HL * c, INTER + CONV_DIM + HL * (c + 1))
        w1 = np.concatenate([in_proj_w[gsl], in_proj_w[xsl], in_proj_w[bsl],
                             in_proj_w[cslc], in_proj_w[dsl]], axis=0)
        w1 = np.concatenate(
            [w1, np.zeros((NF * 128 - w1.shape[0], H_SIZE), np.float32)],
            axis=0)
        # W1T [4096, 2432]: [half, kk, r, f, fc] -> [f, half, r, kk, fc]
        w1f = np.ascontiguousarray(
            w1.T.reshape(2, 16, 128, NF, 128).transpose(3, 0, 2, 1, 4)
            .reshape(NF, 2, 128, 16 * 128))
        w2 = out_proj_w[:, gsl] * norm_w[gsl][None, :]  # norm_w folded
        # W2T [1024, 4096]: [kt, r, m, mc] -> [m, r, kt, mc]
        w2m = np.ascontiguousarray(
            w2.T.reshape(NK2, 128, NM2, 128).transpose(2, 1, 0, 3)
            .reshape(NM2, 128, NK2 * 128))
        conv_idx = np.concatenate([
            np.arange(CLOC * c, CLOC * (c + 1)),
            np.arange(INTER + SS * c, INTER + SS * (c + 1)),
            np.arange(INTER + NG * SS + SS * c,
                      INTER + NG * SS + SS * (c + 1))])
        cwl = conv_w[conv_idx, 0, :]          # [1280, 4]
        cbl = conv_b[conv_idx]                # [1280]
        convw = np.ascontiguousarray(
            cwl.reshape(10, 128, KCONV).transpose(1, 0, 2)
            .reshape(128, 10 * KCONV))
        convb = np.ascontiguousarray(cbl.reshape(10, 128).transpose(1, 0))
        hsl = slice(HL * c, HL * (c + 1))
        acol = -(np.arange(HL * c + 1, HL * (c + 1) + 1, dtype=np.float32))
        in_maps.append({
            "hids": hids,
            "w1f": w1f,
            "w2m": w2m,
            "convw": convw,
            "convb": convb,
            "dtbias": dt_bias[hsl].reshape(HL, 1).astype(np.float32),
            "acol": acol.reshape(HL, 1),
            "dbc": np.tile(D[hsl][None, :], (128, 1)).astype(np.float32),
            "negmask": negmask,
            "ident": ident,
            "e127": e127,
        })
    return in_maps


def get_nc():
    if "nc" not in _CACHE:
        _CACHE["nc"] = build_nc()
    return _CACHE["nc"]


def kernel(hidden_states, in_proj_w, conv_w, conv_b, dt_bias, D, norm_w,
           out_proj_w):
    nc = get_nc()
    in_maps = prepare_in_maps(
        np.asarray(hidden_states, np.float32),
        np.asarray(in_proj_w, np.float32),
        np.asarray(conv_w, np.float32), np.asarray(conv_b, np.float32),
        np.asarray(dt_bias, np.float32), np.asarray(D, np.float32),
        np.asarray(norm_w, np.float32), np.asarray(out_proj_w, np.float32))
    res = run_bass_kernel_spmd(nc, in_maps, list(range(N_CORES)))
    acc = np.zeros((H_SIZE, S), np.float64)
    for r in res.results:
        acc += r["outp"].transpose(0, 2, 1, 3).reshape(H_SIZE, S)
    return acc.T.astype(np.float32).reshape(1, S, H_SIZE)



# revision 23
# speedup vs baseline: 1.7276x; 1.7276x over previous
"""NemotronH Mamba2 mixer on 8 Trainium2 cores (Bass/Tile).

Sharding: tensor-parallel over heads/groups. Core c owns group c =
16 heads (= 1024 gate/x channels, 128 B + 128 C state channels, 16 dt).
in_proj rows and out_proj columns are sharded accordingly; out_proj is
row-parallel over the contraction, partials are combined on the host.

v2: bf16 matmul operands everywhere (fp32 kept on the decay/cumsum,
softplus and state-accumulation paths), per-head cs broadcast via
full-rate fp32r matmuls (free dim 256 over chunk pairs), softplus and
rsqrt via the exp+ln activation table (two table loads per superblock),
conv taps on GPSIMD, per-head scalars applied with stride-0 3D
broadcast APs in single wide DVE ops, bf16 output partials.
"""

import numpy as np
import ml_dtypes

import concourse.bass as bass
import concourse.mybir as mybir
from concourse import bacc
from concourse.tile import TileContext
from concourse.bass_utils import run_bass_kernel_spmd

F32 = mybir.dt.float32
F32R = mybir.dt.float32r
BF16 = mybir.dt.bfloat16
AF = mybir.ActivationFunctionType
ALU = mybir.AluOpType
NPBF16 = ml_dtypes.bfloat16

# Model dims
H_SIZE = 4096
NH = 128
HD = 64
SS = 128
KCONV = 4
NG = 8
CHUNK = 128
INTER = NH * HD                 # 8192
CONV_DIM = INTER + 2 * NG * SS  # 10240
PROJ = INTER + CONV_DIM + NH    # 18560
DT_MIN, DT_MAX = 0.001, 100.0
EPS = 1e-5
GROUP = INTER // NG             # 1024

# Sharding / tiling
N_CORES = 8
S = 2048
HL = NH // N_CORES              # 16 local heads
CLOC = HL * HD                  # 1024 local gate/x channels
NSB = 4                         # seq superblocks
SB = S // NSB                   # 512
NCPB = SB // CHUNK              # 4 chunks per superblock
NF = 19                         # f-tiles: 8 gate + 8 x + B + C + dt
NK1 = H_SIZE // 128             # 32 k-tiles for in_proj
NK2 = CLOC // 128               # 8 k-tiles for out_proj
NM2 = H_SIZE // 128             # 32 m-tiles for out_proj

_CACHE = {}


def r32(ap):
    return ap.bitcast(F32R)


def build_nc():
    nc = bacc.Bacc(None, target_bir_lowering=False)

    # hidden, pre-tiled: [sb, half, 128, 16*512] bf16 (chan-major k-tiles)
    hids = nc.declare_dram_parameter("hids", [NSB, 2, 128, 16 * SB], BF16,
                                     isOutput=False)
    # in_proj weights per f-tile: [f, half, 128, 16*128] bf16
    w1f = nc.declare_dram_parameter("w1f", [NF, 2, 128, 16 * 128], BF16,
                                    isOutput=False)
    # out_proj weights, groups of 4 m-tiles: [m/4, 128, 4*8*128] bf16
    w2m = nc.declare_dram_parameter("w2m", [NM2 // 4, 128, 4 * NK2 * 128],
                                    BF16, isOutput=False)
    convw = nc.declare_dram_parameter("convw", [128, 10 * KCONV], F32,
                                      isOutput=False)
    convb = nc.declare_dram_parameter("convb", [128, 10], F32, isOutput=False)
    dtbias = nc.declare_dram_parameter("dtbias", [HL, 1], F32, isOutput=False)
    acol = nc.declare_dram_parameter("acol", [HL, 1], F32, isOutput=False)
    dbc = nc.declare_dram_parameter("dbc", [128, HL], F32, isOutput=False)
    idf = nc.declare_dram_parameter("idf", [128, 128], F32, isOutput=False)
    idb = nc.declare_dram_parameter("idb", [128, 128], BF16, isOutput=False)
    trim = nc.declare_dram_parameter("trim", [128, 128], BF16, isOutput=False)
    outp = nc.declare_dram_parameter("outp", [NM2, NSB, 128, SB], F32,
                                     isOutput=True)

    with TileContext(nc) as tc:
        with tc.tile_pool(name="const", bufs=1) as cp:
            idf_sb = cp.tile([128, 128], F32, tag="idf")
            idr_sb = cp.tile([128, 128], F32R, tag="idr")
            idb_sb = cp.tile([128, 128], BF16, tag="idb")
            trim_sb = cp.tile([128, 128], BF16, tag="trim")
            cw_sb = cp.tile([128, 10 * KCONV], F32, tag="cw")
            cb_sb = cp.tile([128, 10], F32, tag="cb")
            dtb_sb = cp.tile([HL, 1], F32, tag="dtb")
            a_sb = cp.tile([HL, 1], F32, tag="acol")
            dbc_sb = cp.tile([128, HL], F32, tag="dbc")
            ones16 = cp.tile([HL, CHUNK], F32, tag="ones16")
            zcol = cp.tile([128, 1], F32, tag="zcol")
            st_sb = cp.tile([128, HL * HD], F32R, tag="state")
            nc.sync.dma_start(out=idf_sb[:], in_=idf[:])
            nc.sync.dma_start(out=idr_sb[:], in_=r32(idf[:]))
            nc.sync.dma_start(out=idb_sb[:], in_=idb[:])
            nc.sync.dma_start(out=trim_sb[:], in_=trim[:])
            nc.sync.dma_start(out=cw_sb[:], in_=convw[:])
            nc.sync.dma_start(out=cb_sb[:], in_=convb[:])
            nc.sync.dma_start(out=dtb_sb[:], in_=dtbias[:])
            nc.sync.dma_start(out=a_sb[:], in_=acol[:])
            nc.sync.dma_start(out=dbc_sb[:], in_=dbc[:])
            nc.vector.memset(ones16[:], 1.0)
            nc.vector.memset(zcol[:], 0.0)

            _main_phase(nc, tc, hids, w1f, w2m, outp,
                        idf_sb, idr_sb, idb_sb, trim_sb, cw_sb, cb_sb,
                        dtb_sb, a_sb, dbc_sb, ones16, zcol, st_sb)

    nc.compile()
    return nc


def _main_phase(nc, tc, hids, w1f, w2m, outp,
                idf_sb, idr_sb, idb_sb, trim_sb, cw_sb, cb_sb,
                dtb_sb, a_sb, dbc_sb, ones16, zcol, st_sb):
    with tc.tile_pool(name="hid", bufs=1) as hidp, \
         tc.tile_pool(name="w1", bufs=3) as w1p, \
         tc.tile_pool(name="w2", bufs=2) as w2p, \
         tc.tile_pool(name="stage", bufs=1) as sgp, \
         tc.tile_pool(name="qst", bufs=2) as qstp, \
         tc.tile_pool(name="conv32", bufs=2) as cvp, \
         tc.tile_pool(name="pair", bufs=1) as prp, \
         tc.tile_pool(name="seg", bufs=2) as segp, \
         tc.tile_pool(name="ch", bufs=2) as chp, \
         tc.tile_pool(name="ch1", bufs=1) as ch1p, \
         tc.tile_pool(name="oev", bufs=2) as oevp, \
         tc.tile_pool(name="acc", bufs=2, space="PSUM") as accp, \
         tc.tile_pool(name="psY", bufs=2, space="PSUM") as psY, \
         tc.tile_pool(name="psPB", bufs=2, space="PSUM") as psPB, \
         tc.tile_pool(name="psT", bufs=2, space="PSUM") as psT:

        # conv input staging: 10 channel tiles (8 x, 1 B, 1 C), 3 halo + SB
        ccat = sgp.tile([128, 10 * (SB + 3)], F32, tag="ccat")
        for t in range(10):
            nc.vector.memset(ccat[:, t * (SB + 3):t * (SB + 3) + 3], 0.0)

        pending_out = []

        w2cache = {}

        def emit_outproj(m, qst, sbq):
            G, g = m // 4, m % 4
            if w2cache.get("G") != (G, sbq):
                w2 = w2p.tile([128, 4 * NK2 * 128], BF16, tag="w2")
                nc.sync.dma_start(out=w2[:], in_=w2m[G])
                w2cache["G"] = (G, sbq)
                w2cache["t"] = w2
            w2 = w2cache["t"]
            acc = accp.tile([128, SB], F32, tag="acc")
            for kt in range(NK2):
                nc.tensor.matmul(
                    acc[:], w2[:, (g * NK2 + kt) * 128:(g * NK2 + kt + 1) * 128],
                    qst[:, kt * SB:kt * SB + SB],
                    start=(kt == 0), stop=(kt == NK2 - 1))
            ev = oevp.tile([128, SB], F32, tag="oev")
            nc.scalar.copy(ev[:], acc[:])
            nc.sync.dma_start(out=outp[m, sbq], in_=ev[:])

        for sb in range(NSB):
            quarters = []
            for kq in range(8):
                hid_q = hidp.tile([128, 4 * SB], BF16, tag=f"hid{kq}",
                                  name=f"hid{kq}")
                nc.sync.dma_start(
                    out=hid_q[:],
                    in_=hids[sb, kq // 4][:, (kq % 4) * 4 * SB:
                                          (kq % 4 + 1) * 4 * SB])
                quarters.append(hid_q)

            gate_sb = sgp.tile([128, 8 * SB], BF16, tag="gate")
            dtraw = sgp.tile([HL, SB], F32, tag="dtraw")

            # halo copies must read previous superblock before overwrite
            if sb > 0:
                for t in range(10):
                    base = t * (SB + 3)
                    nc.vector.tensor_copy(
                        ccat[:, base:base + 3], ccat[:, base + SB:base + SB + 3])

            def emit_ftile(f, gate_sb=gate_sb, dtraw=dtraw,
                           quarters=quarters):
                w1h = []
                for khalf in range(2):
                    w1t_ = w1p.tile([128, 16 * 128], BF16, tag="w1")
                    nc.sync.dma_start(out=w1t_[:], in_=w1f[f, khalf])
                    w1h.append(w1t_)
                acc = accp.tile([128, SB], F32, tag="acc")
                for k in range(NK1):
                    nc.tensor.matmul(
                        acc[:],
                        w1h[k // 16][:, (k % 16) * 128:(k % 16 + 1) * 128],
                        quarters[k // 4][:, (k % 4) * SB:(k % 4 + 1) * SB],
                        start=(k == 0), stop=(k == NK1 - 1))
                if f < 8:
                    nc.scalar.copy(gate_sb[:, f * SB:(f + 1) * SB], acc[:])
                elif f < 18:
                    t = f - 8
                    base = t * (SB + 3)
                    nc.scalar.copy(ccat[:, base + 3:base + 3 + SB], acc[:])
                else:
                    nc.scalar.copy(dtraw[:, :], acc[:HL, :])

            drain = list(pending_out)
            pending_out.clear()
            for u in [18, 16, 17] + list(range(8, 16)) + list(range(8)):
                emit_ftile(u)

            # ---------------- dt pipeline (exp/ln softplus) ----------------
            # z = dtraw + dt_bias; sp = relu(z) + ln(1 + exp(-|z|))
            az = sgp.tile([HL, SB], F32, tag="az")
            dtsp = dtraw  # in-place: relu(z)+ln1p overwrites raw dt
            cs = sgp.tile([HL, SB], F32, tag="cs")
            nc.scalar.activation(az[:], dtraw[:], AF.Abs, bias=dtb_sb[:, 0:1])
            nc.scalar.activation(az[:], az[:], AF.Exp, scale=-1.0)
            nc.vector.tensor_scalar(az[:], az[:], 1.0, None, ALU.add)
            nc.scalar.activation(az[:], az[:], AF.Ln)
            nc.scalar.activation(dtsp[:], dtraw[:], AF.Relu,
                                 bias=dtb_sb[:, 0:1])
            nc.vector.tensor_tensor(dtsp[:], dtsp[:], az[:], ALU.add)
            nc.vector.tensor_scalar(dtsp[:], dtsp[:], DT_MIN, DT_MAX,
                                    ALU.max, ALU.min)
            dA = az  # az dead, reuse
            nc.vector.tensor_scalar(dA[:], dtsp[:], a_sb[:, 0:1], None,
                                    ALU.mult)
            for cl in range(NCPB):
                nc.vector.tensor_tensor_scan(
                    cs[:, cl * CHUNK:(cl + 1) * CHUNK],
                    ones16[:], dA[:, cl * CHUNK:(cl + 1) * CHUNK],
                    0.0, ALU.mult, ALU.add)

            # SiLU on gate (batched with conv SiLUs: one table load)
            nc.scalar.activation(gate_sb[:], gate_sb[:], AF.Silu)

            # ---------------- conv taps (GPSIMD) + SiLU ----------------
            xcs = sgp.tile([128, 8 * SB], F32R, tag="xcs")
            bcs = sgp.tile([128, SB], F32R, tag="bcs")
            ccs = sgp.tile([128, SB], F32R, tag="ccs")
            for t in [8, 9] + list(range(8)):
                base = t * (SB + 3)
                eng = nc.vector
                c32 = cvp.tile([128, SB], F32, tag="c32")
                eng.tensor_scalar(
                    c32[:], ccat[:, base:base + SB],
                    cw_sb[:, t * KCONV:t * KCONV + 1], cb_sb[:, t:t + 1],
                    ALU.mult, ALU.add)
                for j in range(1, KCONV):
                    eng.scalar_tensor_tensor(
                        c32[:], ccat[:, base + j:base + j + SB],
                        cw_sb[:, t * KCONV + j:t * KCONV + j + 1], c32[:],
                        ALU.mult, ALU.add)
                dst = (xcs[:, t * SB:(t + 1) * SB] if t < 8
                       else (bcs[:] if t == 8 else ccs[:]))
                nc.scalar.activation(dst, c32[:], AF.Silu)

            # csT/dtT for all chunks: [128, cl*HL + h] / [128, (4+cl)*HL + h]
            pcs = psPB.tile([128, 2 * NCPB * HL], F32, tag="pb",
                            name="pcs")
            for cl in range(NCPB):
                nc.tensor.transpose(
                    pcs[:, cl * HL:(cl + 1) * HL],
                    cs[:, cl * CHUNK:(cl + 1) * CHUNK], idf_sb[:HL, :HL])
                nc.tensor.transpose(
                    pcs[:, (NCPB + cl) * HL:(NCPB + cl + 1) * HL],
                    dtsp[:, cl * CHUNK:(cl + 1) * CHUNK], idf_sb[:HL, :HL])
            csdtT = sgp.tile([128, 2 * NCPB * HL], F32, tag="csdtT")
            nc.scalar.copy(csdtT[:], pcs[:])
            negcsT = sgp.tile([128, NCPB * HL], F32, tag="negcsT")
            nc.vector.tensor_scalar(negcsT[:], csdtT[:, :NCPB * HL], -1.0,
                                    None, ALU.mult)

            qstage = qstp.tile([128, NK2 * SB], BF16, tag="qstage")
            ygat = sgp.tile([128, NCPB * CLOC], BF16, tag="ygat")
            ssum = sgp.tile([128, NCPB], F32, tag="ssum")

            # ---------------- SSD chunk pairs ----------------
            for pr in range(NCPB // 2):
                prsl = slice(pr * 2 * CHUNK, (pr + 1) * 2 * CHUNK)
                # per-head cs broadcast: pb[p, j*256+l] = cs[h, pr*256+l]
                # (fp32r, free 256 -> full rate), 2 heads per PSUM tile
                epb = prp.tile([128, HL * 2 * CHUNK], F32R, tag="epb")
                segs = [segp.tile([128, HL * CHUNK], F32R, tag="seg",
                                  name=f"seg{i}")
                        for i in range(2)]
                for hg in range(HL // 2):
                    pb = psPB.tile([128, 512], F32, tag="pb")
                    for j in range(2):
                        h = 2 * hg + j
                        idcol = idf_sb[:HL, h:h + 1]
                        indh = bass.AP(tensor=idcol.tensor,
                                       offset=idcol.offset,
                                       ap=[[idcol.ap[0][0], HL], [0, 128]])
                        nc.tensor.matmul(pb[:, j * 256:(j + 1) * 256],
                                         indh, cs[:, prsl],
                                         start=True, stop=True)
                    # seg[s, l] = min(cs[h,l] - cs[h,s], 0) per chunk
                    for lc in range(2):
                        cl = 2 * pr + lc
                        for j in range(2):
                            h = 2 * hg + j
                            nc.vector.scalar_tensor_tensor(
                                segs[lc][:, h * CHUNK:(h + 1) * CHUNK],
                                pb[:, j * 256 + lc * 128:
                                   j * 256 + (lc + 1) * 128],
                                negcsT[:, cl * HL + h:cl * HL + h + 1],
                                bass.AP(tensor=zcol.tensor,
                                        offset=zcol[:].offset,
                                        ap=[zcol[:].ap[0], [0, CHUNK]]),
                                ALU.add, ALU.min)
                    nc.scalar.activation(epb[:, hg * 512:(hg + 1) * 512],
                                         pb[:], AF.Exp)

                for lc in range(2):
                    cl = 2 * pr + lc
                    _emit_chunk(nc, sb * NCPB + cl, cl, lc, gate_sb, xcs,
                                bcs, ccs,
                                csdtT, segs[lc], epb, ygat, ssum,
                                idf_sb, idr_sb, idb_sb, trim_sb, dbc_sb,
                                st_sb, chp, ch1p, psY, psT, psPB)
                    for _ in range(min(8, len(drain))):
                        emit_outproj(*drain.pop(0))

            # ---------------- deferred group RMSNorm + transposes ----------
            lnm = chp.tile([128, NCPB], F32, tag="lnm")
            rstd = chp.tile([128, NCPB], F32, tag="rstd")
            nc.vector.tensor_scalar(lnm[:], ssum[:], 1.0 / GROUP, EPS,
                                    ALU.mult, ALU.add)
            nc.scalar.activation(lnm[:], lnm[:], AF.Ln)
            nc.scalar.activation(rstd[:], lnm[:], AF.Exp, scale=-0.5)
            for cl in range(NCPB):
                normed = ch1p.tile([128, CLOC], BF16, tag="normed")
                nc.vector.tensor_scalar(
                    normed[:], ygat[:, cl * CLOC:(cl + 1) * CLOC],
                    rstd[:, cl:cl + 1], None, ALU.mult)
                nps = psT.tile([128, CLOC], BF16, tag="trans")
                for t in range(NK2):
                    nc.tensor.transpose(
                        nps[:, t * 128:(t + 1) * 128],
                        normed[:, t * 128:(t + 1) * 128], idb_sb[:])
                qdst = qstage[:].rearrange(
                    "p (t s) -> p t s", t=NK2)[:, :, cl * 128:(cl + 1) * 128]
                nsrc = nps[:].rearrange("p (t s) -> p t s", t=NK2)
                nc.scalar.copy(qdst, nsrc)

            while drain:
                emit_outproj(*drain.pop(0))
            pending_out.extend((m, qstage, sb) for m in range(NM2))

        while pending_out:
            emit_outproj(*pending_out.pop(0))


def _emit_chunk(nc, gc, cl, lc, gate_sb, xcs, bcs, ccs,
                csdtT, seg, epb, ygat, ssum,
                idf_sb, idr_sb, idb_sb, trim_sb, dbc_sb,
                st_sb, chp, ch1p, psY, psT, psPB):
    csl = slice(cl * CHUNK, (cl + 1) * CHUNK)

    # gate transpose (silu already applied) -> silg_c
    gps = psT.tile([128, CLOC], BF16, tag="trans")
    for t in range(8):
        nc.tensor.transpose(
            gps[:, t * 128:(t + 1) * 128],
            gate_sb[:, t * SB + cl * CHUNK:t * SB + (cl + 1) * CHUNK],
            idb_sb[:])
    silg = chp.tile([128, CLOC], BF16, tag="silg")
    nc.scalar.copy(silg[:], gps[:])

    # scores = exp(seg) * (triu-in-[s,l] . gram); gram^T = B C^T in [s, l]
    gram_ps = psPB.tile([128, 128], F32, tag="pb", name="gram_ps")
    nc.tensor.matmul(gram_ps[:], bcs[:, csl], ccs[:, csl],
                     start=True, stop=True)
    gram = chp.tile([128, 128], F32, tag="gramm")
    nc.vector.tensor_tensor(gram[:], gram_ps[:], trim_sb[:], ALU.mult)

    # chunk-end decay per head: cend = exp(cs_end), decT = exp(cs_end - cs)
    # (both extracted BEFORE seg/epb are overwritten in place below)
    cend = chp.tile([128, HL], F32, tag="cend")
    ep1 = epb[:, (lc + 1) * CHUNK - 1:(lc + 1) * CHUNK]
    epb_end = bass.AP(tensor=epb.tensor, offset=ep1.offset,
                      ap=[ep1.ap[0], [2 * CHUNK, HL]])
    nc.vector.tensor_copy(cend[:], epb_end)
    decT = chp.tile([128, HL], F32, tag="decT")
    # seg column l=CHUNK-1 holds cs_end - cs[s] (<=0, min-clamp no-op there)
    sg1 = seg[:, CHUNK - 1:CHUNK]
    seg_end = bass.AP(tensor=seg.tensor, offset=sg1.offset,
                      ap=[sg1.ap[0], [CHUNK, HL]])
    nc.scalar.activation(decT[:], seg_end, AF.Exp)

    # scores = exp(seg) * gram, in place on seg
    scores = seg
    nc.scalar.activation(scores[:], seg[:], AF.Exp)
    s3 = scores[:].rearrange("p (h l) -> p h l", h=HL)
    gram_b = bass.AP(tensor=gram.tensor, offset=gram[:].offset,
                     ap=[gram[:].ap[0], [0, HL], [1, 128]])
    nc.vector.tensor_tensor(s3, s3, gram_b, ALU.mult)

    # e4 = exp(pb) * C (for Yoff), in place on this chunk's epb columns
    ep0 = epb[:, lc * CHUNK:lc * CHUNK + 1]
    epb_3 = bass.AP(tensor=epb.tensor, offset=ep0.offset,
                    ap=[ep0.ap[0], [2 * CHUNK, HL], [1, CHUNK]])
    cc0 = ccs[:, cl * CHUNK:cl * CHUNK + 1]
    ccs_b = bass.AP(tensor=ccs.tensor, offset=cc0.offset,
                    ap=[cc0.ap[0], [0, HL], [1, CHUNK]])
    nc.vector.tensor_tensor(epb_3, epb_3, ccs_b, ALU.mult)
    ddt = chp.tile([128, HL], F32, tag="ddt")
    nc.vector.tensor_tensor(ddt[:], csdtT[:, (NCPB + cl) * HL:
                                           (NCPB + cl + 1) * HL],
                            decT[:], ALU.mult)

    # x transpose -> xT (f32), then xdt / xdd
    xT = ch1p.tile([128, CLOC], F32R, tag="xT")
    for hx in range(2):
        xps = psT.tile([128, 512], F32R, tag="trans", name=f"xps{hx}")
        for t in range(4):
            tt = hx * 4 + t
            nc.tensor.transpose(
                xps[:, t * 128:(t + 1) * 128],
                xcs[:, tt * SB + cl * CHUNK:tt * SB + (cl + 1) * CHUNK],
                idr_sb[:])
        nc.scalar.copy(xT[:, hx * 512:(hx + 1) * 512], xps[:])
    xdt = ch1p.tile([128, CLOC], F32R, tag="xdt")
    x3 = xT[:].rearrange("p (h j) -> p h j", h=HL)
    dt0 = csdtT[:, (NCPB + cl) * HL:(NCPB + cl) * HL + 1]
    dt_b = bass.AP(tensor=csdtT.tensor, offset=dt0.offset,
                   ap=[dt0.ap[0], [1, HL], [0, HD]])
    ddt_b = bass.AP(tensor=ddt.tensor, offset=ddt[:].offset,
                    ap=[ddt[:].ap[0], [1, HL], [0, HD]])
    nc.vector.tensor_tensor(xdt[:].rearrange("p (h j) -> p h j", h=HL),
                            x3, dt_b, ALU.mult)
    # ysb = x*D now (before xdd overwrites xT in place)
    dbc_b = bass.AP(tensor=dbc_sb.tensor, offset=dbc_sb[:].offset,
                    ap=[dbc_sb[:].ap[0], [1, HL], [0, HD]])
    ysb = ch1p.tile([128, CLOC], F32, tag="ysb")
    nc.vector.tensor_tensor(ysb[:].rearrange("p (h j) -> p h j", h=HL),
                            x3, dbc_b, ALU.mult)
    xdd = xT  # in place: x * ddt overwrites xT
    nc.vector.tensor_tensor(xdd[:].rearrange("p (h j) -> p h j", h=HL),
                            x3, ddt_b, ALU.mult)

    # B chunk transposed (bln) for state matmuls
    pbt = psPB.tile([128, 128], F32R, tag="pb", name="pbt")
    nc.tensor.transpose(pbt[:], bcs[:, csl], idr_sb[:])
    bln = chp.tile([128, 128], F32R, tag="bln")
    nc.scalar.copy(bln[:], pbt[:])

    # Ydiag + Yoff accumulated per head (two matmuls per head)
    y_halves = []
    for half in range(2):
        y_ps = psY.tile([128, 512], F32, tag="y", name=f"y{half}")
        for hh in range(8):
            h = half * 8 + hh
            hs = slice(hh * HD, (hh + 1) * HD)
            nc.tensor.matmul(
                y_ps[:, hs], scores[:, h * CHUNK:(h + 1) * CHUNK],
                xdt[:, h * HD:(h + 1) * HD], start=True, stop=(gc == 0))
            if gc > 0:
                nc.tensor.matmul(
                    y_ps[:, hs],
                    epb[:, h * 2 * CHUNK + lc * CHUNK:
                        h * 2 * CHUNK + (lc + 1) * CHUNK],
                    st_sb[:, h * HD:(h + 1) * HD], start=False, stop=True)
        y_halves.append(y_ps)

    # states for this chunk
    s_halves = []
    for half in range(2):
        s_ps = psY.tile([128, 512], F32, tag="y", name=f"s{half}")
        nc.tensor.matmul(
            s_ps[:], bln[:], xdd[:, half * 512:(half + 1) * 512],
            start=True, stop=True)
        s_halves.append(s_ps)

    # y = (Ydiag + Yoff) + D*x, gated; squares accumulated for RMS
    ysl = slice(cl * CLOC, (cl + 1) * CLOC)
    for half in range(2):
        hsl = slice(half * 512, (half + 1) * 512)
        nc.vector.tensor_tensor(ysb[:, hsl], ysb[:, hsl],
                                y_halves[half][:], ALU.add)
    nc.vector.tensor_tensor(ysb[:], ysb[:], silg[:], ALU.mult)
    nc.scalar.copy(ygat[:, ysl], ysb[:])
    nc.scalar.activation(xdt[:], ysb[:], AF.Square,
                         accum_out=ssum[:, cl:cl + 1])

    # state update: st = st * exp(cs_end) + s  (first chunk: st = s)
    if gc == 0:
        for half in range(2):
            hsl = slice(half * 512, (half + 1) * 512)
            nc.vector.tensor_copy(st_sb[:, hsl], s_halves[half][:])
    else:
        cend_b = bass.AP(tensor=cend.tensor, offset=cend[:].offset,
                         ap=[cend[:].ap[0], [1, HL], [0, HD]])
        st3 = st_sb[:].rearrange("p (h j) -> p h j", h=HL)
        nc.vector.tensor_tensor(st3, st3, cend_b, ALU.mult)
        for half in range(2):
            hsl = slice(half * 512, (half + 1) * 512)
            nc.vector.tensor_tensor(st_sb[:, hsl], st_sb[:, hsl],
                                    s_halves[half][:], ALU.add)


def prepare_in_maps(hidden_states, in_proj_w, conv_w, conv_b, dt_bias, D,
                    norm_w, out_proj_w):
    hidT = np.ascontiguousarray(hidden_states.reshape(S, H_SIZE).T)
    # [half, kk, r, sb, c] -> [sb, half, r, kk, c]
    hids = np.ascontiguousarray(
        hidT.reshape(2, 16, 128, NSB, SB).transpose(3, 0, 2, 1, 4)
        .reshape(NSB, 2, 128, 16 * SB)).astype(NPBF16)
    idf = np.eye(128, dtype=np.float32)
    idb = np.eye(128).astype(NPBF16)
    # mask in [s, l]: keep l >= s
    trim = np.triu(np.ones((128, 128), np.float32)).astype(NPBF16)
    in_maps = []
    for c in range(N_CORES):
        gsl = slice(CLOC * c, CLOC * (c + 1))
        xsl = slice(INTER + CLOC * c, INTER + CLOC * (c + 1))
        bsl = slice(2 * INTER + SS * c, 2 * INTER + SS * (c + 1))
        cslc = slice(2 * INTER + NG * SS + SS * c,
                     2 * INTER + NG * SS + SS * (c + 1))
        dsl = slice(INTER + CONV_DIM + HL * c, INTER + CONV_DIM + HL * (c + 1))
        w1 = np.concatenate([in_proj_w[gsl], in_proj_w[xsl], in_proj_w[bsl],
                             in_proj_w[cslc], in_proj_w[dsl]], axis=0)
        w1 = np.concatenate(
            [w1, np.zeros((NF * 128 - w1.shape[0], H_SIZE), np.float32)],
            axis=0)
        w1f = np.ascontiguousarray(
            w1.T.reshape(2, 16, 128, NF, 128).transpose(3, 0, 2, 1, 4)
            .reshape(NF, 2, 128, 16 * 128)).astype(NPBF16)
        w2 = out_proj_w[:, gsl] * norm_w[gsl][None, :]  # norm_w folded
        w2m = np.ascontiguousarray(
            w2.T.reshape(NK2, 128, NM2, 128).transpose(2, 1, 0, 3)
            .reshape(NM2 // 4, 4 * 128 * NK2 * 128 // 1024, 1024)
            .reshape(NM2 // 4, 4, 128, NK2 * 128).transpose(0, 2, 1, 3)
            .reshape(NM2 // 4, 128, 4 * NK2 * 128)).astype(NPBF16)
        conv_idx = np.concatenate([
            np.arange(CLOC * c, CLOC * (c + 1)),
            np.arange(INTER + SS * c, INTER + SS * (c + 1)),
            np.arange(INTER + NG * SS + SS * c,
                      INTER + NG * SS + SS * (c + 1))])
        cwl = conv_w[conv_idx, 0, :]          # [1280, 4]
        cbl = conv_b[conv_idx]                # [1280]
        convw = np.ascontiguousarray(
            cwl.reshape(10, 128, KCONV).transpose(1, 0, 2)
            .reshape(128, 10 * KCONV))
        convb = np.ascontiguousarray(cbl.reshape(10, 128).transpose(1, 0))
        hsl = slice(HL * c, HL * (c + 1))
        acol = -(np.arange(HL * c + 1, HL * (c + 1) + 1, dtype=np.float32))
        in_maps.append({
            "hids": hids,
            "w1f": w1f,
            "w2m": w2m,
            "convw": convw,
            "convb": convb,
            "dtbias": dt_bias[hsl].reshape(HL, 1).astype(np.float32),
            "acol": acol.reshape(HL, 1),
            "dbc": np.tile(D[hsl][None, :], (128, 1)).astype(np.float32),
            "idf": idf,
            "idb": idb,
            "trim": trim,
        })
    return in_maps


def get_nc():
    if "nc" not in _CACHE:
        _CACHE["nc"] = build_nc()
    return _CACHE["nc"]


def kernel(hidden_states, in_proj_w, conv_w, conv_b, dt_bias, D, norm_w,
           out_proj_w):
    nc = get_nc()
    in_maps = prepare_in_maps(
        np.asarray(hidden_states, np.float32),
        np.asarray(in_proj_w, np.float32),
        np.asarray(conv_w, np.float32), np.asarray(conv_b, np.float32),
        np.asarray(dt_bias, np.float32), np.asarray(D, np.float32),
        np.asarray(norm_w, np.float32), np.asarray(out_proj_w, np.float32))
    res = run_bass_kernel_spmd(nc, in_maps, list(range(N_CORES)))
    acc = np.zeros((H_SIZE, S), np.float64)
    for r in res.results:
        acc += np.asarray(r["outp"], np.float64).transpose(0, 2, 1, 3) \
                 .reshape(H_SIZE, S)
    return acc.T.astype(np.float32).reshape(1, S, H_SIZE)


# revision 27
# speedup vs baseline: 1.7879x; 1.0349x over previous
"""NemotronH Mamba2 mixer on 8 Trainium2 cores (Bass/Tile).

Sharding: tensor-parallel over heads/groups. Core c owns group c =
16 heads (= 1024 gate/x channels, 128 B + 128 C state channels, 16 dt).
in_proj rows and out_proj columns are sharded accordingly; out_proj is
row-parallel over the contraction, partials are combined on the host.

v2: bf16 matmul operands everywhere (fp32 kept on the decay/cumsum,
softplus and state-accumulation paths), per-head cs broadcast via
full-rate fp32r matmuls (free dim 256 over chunk pairs), softplus and
rsqrt via the exp+ln activation table (two table loads per superblock),
conv taps on GPSIMD, per-head scalars applied with stride-0 3D
broadcast APs in single wide DVE ops, bf16 output partials.
"""

import numpy as np
import ml_dtypes

import concourse.bass as bass
import concourse.mybir as mybir
from concourse import bacc
from concourse.tile import TileContext
from concourse.bass_utils import run_bass_kernel_spmd

F32 = mybir.dt.float32
F32R = mybir.dt.float32r
BF16 = mybir.dt.bfloat16
AF = mybir.ActivationFunctionType
ALU = mybir.AluOpType
NPBF16 = ml_dtypes.bfloat16

# Model dims
H_SIZE = 4096
NH = 128
HD = 64
SS = 128
KCONV = 4
NG = 8
CHUNK = 128
INTER = NH * HD                 # 8192
CONV_DIM = INTER + 2 * NG * SS  # 10240
PROJ = INTER + CONV_DIM + NH    # 18560
DT_MIN, DT_MAX = 0.001, 100.0
EPS = 1e-5
GROUP = INTER // NG             # 1024

# Sharding / tiling
N_CORES = 8
S = 2048
HL = NH // N_CORES              # 16 local heads
CLOC = HL * HD                  # 1024 local gate/x channels
NSB = 4                         # seq superblocks
SB = S // NSB                   # 512
NCPB = SB // CHUNK              # 4 chunks per superblock
NF = 19                         # f-tiles: 8 gate + 8 x + B + C + dt
NK1 = H_SIZE // 128             # 32 k-tiles for in_proj
NK2 = CLOC // 128               # 8 k-tiles for out_proj
NM2 = H_SIZE // 128             # 32 m-tiles for out_proj

_CACHE = {}


def r32(ap):
    return ap.bitcast(F32R)


def build_nc():
    nc = bacc.Bacc(None, target_bir_lowering=False)

    # hidden, pre-tiled: [sb, half, 128, 16*512] bf16 (chan-major k-tiles)
    hids = nc.declare_dram_parameter("hids", [NSB, 2, 128, 16 * SB], BF16,
                                     isOutput=False)
    # in_proj weights per f-tile: [f, half, 128, 16*128] bf16
    w1f = nc.declare_dram_parameter("w1f", [NF, 2, 128, 16 * 128], BF16,
                                    isOutput=False)
    # out_proj weights, groups of 4 m-tiles: [m/4, 128, 4*8*128] bf16
    w2m = nc.declare_dram_parameter("w2m", [NM2 // 4, 128, 4 * NK2 * 128],
                                    BF16, isOutput=False)
    convw = nc.declare_dram_parameter("convw", [128, 10 * KCONV], F32,
                                      isOutput=False)
    convb = nc.declare_dram_parameter("convb", [128, 10], F32, isOutput=False)
    dtbias = nc.declare_dram_parameter("dtbias", [HL, 1], F32, isOutput=False)
    acol = nc.declare_dram_parameter("acol", [HL, 1], F32, isOutput=False)
    dbc = nc.declare_dram_parameter("dbc", [128, HL], F32, isOutput=False)
    idf = nc.declare_dram_parameter("idf", [128, 128], F32, isOutput=False)
    idb = nc.declare_dram_parameter("idb", [128, 128], BF16, isOutput=False)
    trim = nc.declare_dram_parameter("trim", [128, 128], BF16, isOutput=False)
    outp = nc.declare_dram_parameter("outp", [NM2, NSB, 128, SB], BF16,
                                     isOutput=True)

    with TileContext(nc) as tc:
        with tc.tile_pool(name="const", bufs=1) as cp:
            idf_sb = cp.tile([128, 128], F32, tag="idf")
            idr_sb = cp.tile([128, 128], F32R, tag="idr")
            idb_sb = cp.tile([128, 128], BF16, tag="idb")
            trim_sb = cp.tile([128, 128], BF16, tag="trim")
            cw_sb = cp.tile([128, 10 * KCONV], F32, tag="cw")
            cb_sb = cp.tile([128, 10], F32, tag="cb")
            dtb_sb = cp.tile([HL, 1], F32, tag="dtb")
            a_sb = cp.tile([HL, 1], F32, tag="acol")
            dbc_sb = cp.tile([128, HL], F32, tag="dbc")
            ones16 = cp.tile([HL, 1], F32, tag="ones16")
            zcol = cp.tile([128, 1], F32, tag="zcol")
            st_sb = cp.tile([128, HL * HD], F32R, tag="state")
            stT = cp.tile([128, HL * HD], BF16, tag="stateb")
            nc.sync.dma_start(out=idf_sb[:], in_=idf[:])
            nc.sync.dma_start(out=idr_sb[:], in_=r32(idf[:]))
            nc.sync.dma_start(out=idb_sb[:], in_=idb[:])
            nc.sync.dma_start(out=trim_sb[:], in_=trim[:])
            nc.sync.dma_start(out=cw_sb[:], in_=convw[:])
            nc.sync.dma_start(out=cb_sb[:], in_=convb[:])
            nc.sync.dma_start(out=dtb_sb[:], in_=dtbias[:])
            nc.sync.dma_start(out=a_sb[:], in_=acol[:])
            nc.sync.dma_start(out=dbc_sb[:], in_=dbc[:])
            nc.vector.memset(ones16[:], 1.0)
            nc.vector.memset(zcol[:], 0.0)

            _main_phase(nc, tc, hids, w1f, w2m, outp,
                        idf_sb, idr_sb, idb_sb, trim_sb, cw_sb, cb_sb,
                        dtb_sb, a_sb, dbc_sb, ones16, zcol, st_sb, stT)

    nc.compile()
    return nc


def _main_phase(nc, tc, hids, w1f, w2m, outp,
                idf_sb, idr_sb, idb_sb, trim_sb, cw_sb, cb_sb,
                dtb_sb, a_sb, dbc_sb, ones16, zcol, st_sb, stT):
    with tc.tile_pool(name="hid", bufs=1) as hidp, \
         tc.tile_pool(name="w1", bufs=3) as w1p, \
         tc.tile_pool(name="w2", bufs=2) as w2p, \
         tc.tile_pool(name="stage", bufs=1) as sgp, \
         tc.tile_pool(name="qst", bufs=2) as qstp, \
         tc.tile_pool(name="conv32", bufs=2) as cvp, \
         tc.tile_pool(name="pair", bufs=1) as prp, \
         tc.tile_pool(name="seg", bufs=2) as segp, \
         tc.tile_pool(name="ch", bufs=2) as chp, \
         tc.tile_pool(name="ch1", bufs=1) as ch1p, \
         tc.tile_pool(name="oev", bufs=2) as oevp, \
         tc.tile_pool(name="acc", bufs=2, space="PSUM") as accp, \
         tc.tile_pool(name="psY", bufs=2, space="PSUM") as psY, \
         tc.tile_pool(name="psPB", bufs=2, space="PSUM") as psPB, \
         tc.tile_pool(name="psT", bufs=2, space="PSUM") as psT:

        # conv input staging: 10 channel tiles (8 x, 1 B, 1 C), 3 halo + SB
        ccat = sgp.tile([128, 10 * (SB + 3)], F32, tag="ccat")
        for t in range(10):
            nc.vector.memset(ccat[:, t * (SB + 3):t * (SB + 3) + 3], 0.0)

        pending_out = []

        w2cache = {}

        def load_w2(G):
            if w2cache.get("G") == G:
                return w2cache["t"]
            w2 = w2p.tile([128, 4 * NK2 * 128], BF16, tag="w2")
            nc.sync.dma_start(out=w2[:], in_=w2m[G])
            w2cache["G"] = G
            w2cache["t"] = w2
            w2cache.setdefault("loaded", set()).add(G)
            return w2

        def emit_outproj(m, qst, sbq):
            G, g = m // 4, m % 4
            if w2cache.get("G") != G:
                # use prefetched tile if the ring already holds it
                w2 = w2cache.get("pref") if w2cache.get("prefG") == G else None
                if w2 is None:
                    w2 = w2p.tile([128, 4 * NK2 * 128], BF16, tag="w2")
                    nc.sync.dma_start(out=w2[:], in_=w2m[G])
                w2cache["G"] = G
                w2cache["t"] = w2
                w2cache["pref"] = None
            w2 = w2cache["t"]
            if g == 0 and G + 1 < NM2 // 4 and w2cache.get("prefG") != G + 1:
                pw = w2p.tile([128, 4 * NK2 * 128], BF16, tag="w2",
                              name="w2pref")
                nc.sync.dma_start(out=pw[:], in_=w2m[G + 1])
                w2cache["prefG"] = G + 1
                w2cache["pref"] = pw
            acc = accp.tile([128, SB], F32, tag="acc")
            for kt in range(NK2):
                nc.tensor.matmul(
                    acc[:], w2[:, (g * NK2 + kt) * 128:(g * NK2 + kt + 1) * 128],
                    qst[:, kt * SB:kt * SB + SB],
                    start=(kt == 0), stop=(kt == NK2 - 1))
            ev = oevp.tile([128, SB], BF16, tag="oev")
            nc.scalar.copy(ev[:], acc[:])
            nc.sync.dma_start(out=outp[m, sbq], in_=ev[:])

        for sb in range(NSB):
            quarters = []
            for kq in range(8):
                hid_q = hidp.tile([128, 4 * SB], BF16, tag=f"hid{kq}",
                                  name=f"hid{kq}")
                nc.sync.dma_start(
                    out=hid_q[:],
                    in_=hids[sb, kq // 4][:, (kq % 4) * 4 * SB:
                                          (kq % 4 + 1) * 4 * SB])
                quarters.append(hid_q)

            gate_sb = sgp.tile([128, 8 * SB], BF16, tag="gate")
            dtraw = sgp.tile([HL, SB], F32, tag="dtraw")

            # halo copies must read previous superblock before overwrite
            if sb > 0:
                for t in range(10):
                    base = t * (SB + 3)
                    nc.vector.tensor_copy(
                        ccat[:, base:base + 3], ccat[:, base + SB:base + SB + 3])

            def emit_ftile(f, gate_sb=gate_sb, dtraw=dtraw,
                           quarters=quarters):
                w1h = []
                for khalf in range(2):
                    w1t_ = w1p.tile([128, 16 * 128], BF16, tag="w1")
                    nc.sync.dma_start(out=w1t_[:], in_=w1f[f, khalf])
                    w1h.append(w1t_)
                acc = accp.tile([128, SB], F32, tag="acc")
                for k in range(NK1):
                    nc.tensor.matmul(
                        acc[:],
                        w1h[k // 16][:, (k % 16) * 128:(k % 16 + 1) * 128],
                        quarters[k // 4][:, (k % 4) * SB:(k % 4 + 1) * SB],
                        start=(k == 0), stop=(k == NK1 - 1))
                if f < 8:
                    nc.scalar.copy(gate_sb[:, f * SB:(f + 1) * SB], acc[:])
                elif f < 18:
                    t = f - 8
                    base = t * (SB + 3)
                    nc.scalar.copy(ccat[:, base + 3:base + 3 + SB], acc[:])
                else:
                    nc.scalar.copy(dtraw[:, :], acc[:HL, :])

            drain = list(pending_out)
            pending_out.clear()
            for u in [18, 16, 17] + list(range(8, 16)) + list(range(8)):
                emit_ftile(u)

            # ---------------- dt pipeline (exp/ln softplus) ----------------
            # z = dtraw + dt_bias; sp = relu(z) + ln(1 + exp(-|z|))
            az = sgp.tile([HL, SB], F32, tag="az")
            dtsp = dtraw  # in-place: relu(z)+ln1p overwrites raw dt
            cs = sgp.tile([HL, SB], F32, tag="cs")
            nc.scalar.activation(az[:], dtraw[:], AF.Abs, bias=dtb_sb[:, 0:1])
            nc.scalar.activation(az[:], az[:], AF.Exp, scale=-1.0)
            nc.vector.tensor_scalar(az[:], az[:], 1.0, None, ALU.add)
            nc.scalar.activation(az[:], az[:], AF.Ln)
            nc.scalar.activation(dtsp[:], dtraw[:], AF.Relu,
                                 bias=dtb_sb[:, 0:1])
            nc.vector.tensor_tensor(dtsp[:], dtsp[:], az[:], ALU.add)
            nc.vector.tensor_scalar(dtsp[:], dtsp[:], DT_MIN, DT_MAX,
                                    ALU.max, ALU.min)
            dA = az  # az dead, reuse
            nc.vector.tensor_scalar(dA[:], dtsp[:], a_sb[:, 0:1], None,
                                    ALU.mult)
            for cl in range(NCPB):
                ones_b = bass.AP(tensor=ones16.tensor,
                                 offset=ones16[:].offset,
                                 ap=[ones16[:].ap[0], [0, CHUNK]])
                nc.vector.tensor_tensor_scan(
                    cs[:, cl * CHUNK:(cl + 1) * CHUNK],
                    ones_b, dA[:, cl * CHUNK:(cl + 1) * CHUNK],
                    0.0, ALU.mult, ALU.add)

            # SiLU on gate (batched with conv SiLUs: one table load)
            nc.scalar.activation(gate_sb[:], gate_sb[:], AF.Silu)

            # ---------------- conv taps (GPSIMD) + SiLU ----------------
            xcs = sgp.tile([128, 8 * SB], BF16, tag="xcs")
            bcs = sgp.tile([128, SB], BF16, tag="bcs")
            ccs = sgp.tile([128, SB], BF16, tag="ccs")
            for t in [8, 9] + list(range(8)):
                base = t * (SB + 3)
                eng = nc.vector
                c32 = cvp.tile([128, SB], F32, tag="c32")
                eng.tensor_scalar(
                    c32[:], ccat[:, base:base + SB],
                    cw_sb[:, t * KCONV:t * KCONV + 1], cb_sb[:, t:t + 1],
                    ALU.mult, ALU.add)
                for j in range(1, KCONV):
                    eng.scalar_tensor_tensor(
                        c32[:], ccat[:, base + j:base + j + SB],
                        cw_sb[:, t * KCONV + j:t * KCONV + j + 1], c32[:],
                        ALU.mult, ALU.add)
                dst = (xcs[:, t * SB:(t + 1) * SB] if t < 8
                       else (bcs[:] if t == 8 else ccs[:]))
                nc.scalar.activation(dst, c32[:], AF.Silu)

            # csT/dtT for all chunks: [128, cl*HL + h] / [128, (4+cl)*HL + h]
            pcs = psPB.tile([128, 2 * NCPB * HL], F32, tag="pb",
                            name="pcs")
            for cl in range(NCPB):
                nc.tensor.transpose(
                    pcs[:, cl * HL:(cl + 1) * HL],
                    cs[:, cl * CHUNK:(cl + 1) * CHUNK], idf_sb[:HL, :HL])
                nc.tensor.transpose(
                    pcs[:, (NCPB + cl) * HL:(NCPB + cl + 1) * HL],
                    dtsp[:, cl * CHUNK:(cl + 1) * CHUNK], idf_sb[:HL, :HL])
            csdtT = sgp.tile([128, 2 * NCPB * HL], F32, tag="csdtT")
            nc.scalar.copy(csdtT[:], pcs[:])
            negcsT = sgp.tile([128, NCPB * HL], F32, tag="negcsT")
            nc.vector.tensor_scalar(negcsT[:], csdtT[:, :NCPB * HL], -1.0,
                                    None, ALU.mult)

            qstage = qstp.tile([128, NK2 * SB], BF16, tag="qstage")
            ygat = sgp.tile([128, NCPB * CLOC], BF16, tag="ygat")
            ssum = sgp.tile([128, NCPB], F32, tag="ssum")

            # ---------------- SSD chunk pairs ----------------
            for pr in range(NCPB // 2):
                prsl = slice(pr * 2 * CHUNK, (pr + 1) * 2 * CHUNK)
                # exact-ish bf16 triple split of cs for full-rate broadcasts
                csh = sgp.tile([HL, 2 * CHUNK], BF16, tag="csh", bufs=1,
                               name="csh")
                csm = sgp.tile([HL, 2 * CHUNK], BF16, tag="csm", bufs=1,
                               name="csm")
                csl_ = sgp.tile([HL, 2 * CHUNK], BF16, tag="csl", bufs=1,
                               name="csl")
                res = sgp.tile([HL, 2 * CHUNK], F32, tag="csres", bufs=1,
                               name="res")
                nc.vector.tensor_copy(csh[:], cs[:, prsl])
                nc.vector.tensor_tensor(res[:], cs[:, prsl], csh[:],
                                        ALU.subtract)
                nc.vector.tensor_copy(csm[:], res[:])
                nc.vector.tensor_tensor(res[:], res[:], csm[:], ALU.subtract)
                nc.vector.tensor_copy(csl_[:], res[:])
                # per-head cs broadcast: pb[p, j*256+l] = cs[h, pr*256+l]
                # (fp32r, free 256 -> full rate), 2 heads per PSUM tile
                epb = prp.tile([128, HL * 2 * CHUNK], F32R, tag="epb")
                segs = [segp.tile([128, HL * CHUNK], F32R, tag="seg",
                                  name=f"seg{i}")
                        for i in range(2)]
                for hg in range(HL // 2):
                    pb = psPB.tile([128, 512], F32, tag="pb")
                    for j in range(2):
                        h = 2 * hg + j
                        idcol = idb_sb[:HL, h:h + 1]
                        indh = bass.AP(tensor=idcol.tensor,
                                       offset=idcol.offset,
                                       ap=[[idcol.ap[0][0], HL], [0, 128]])
                        for si, spl in enumerate((csh, csm, csl_)):
                            nc.tensor.matmul(pb[:, j * 256:(j + 1) * 256],
                                             indh, spl[:],
                                             start=(si == 0), stop=(si == 2))
                    # seg[s, l] = min(cs[h,l] - cs[h,s], 0) per chunk
                    for lc in range(2):
                        cl = 2 * pr + lc
                        for j in range(2):
                            h = 2 * hg + j
                            nc.vector.scalar_tensor_tensor(
                                segs[lc][:, h * CHUNK:(h + 1) * CHUNK],
                                pb[:, j * 256 + lc * 128:
                                   j * 256 + (lc + 1) * 128],
                                negcsT[:, cl * HL + h:cl * HL + h + 1],
                                bass.AP(tensor=zcol.tensor,
                                        offset=zcol[:].offset,
                                        ap=[zcol[:].ap[0], [0, CHUNK]]),
                                ALU.add, ALU.min)
                    nc.scalar.activation(epb[:, hg * 512:(hg + 1) * 512],
                                         pb[:], AF.Exp)

                for lc in range(2):
                    cl = 2 * pr + lc
                    _emit_chunk(nc, sb * NCPB + cl, cl, lc, gate_sb, xcs,
                                bcs, ccs,
                                csdtT, segs[lc], epb, ygat, ssum,
                                idf_sb, idr_sb, idb_sb, trim_sb, dbc_sb,
                                st_sb, stT, chp, ch1p, psY, psT, psPB)
                    for _ in range(min(8, len(drain))):
                        emit_outproj(*drain.pop(0))

            # ---------------- deferred group RMSNorm + transposes ----------
            lnm = chp.tile([128, NCPB], F32, tag="lnm")
            rstd = chp.tile([128, NCPB], F32, tag="rstd")
            nc.vector.tensor_scalar(lnm[:], ssum[:], 1.0 / GROUP, EPS,
                                    ALU.mult, ALU.add)
            nc.scalar.activation(lnm[:], lnm[:], AF.Ln)
            nc.scalar.activation(rstd[:], lnm[:], AF.Exp, scale=-0.5)
            for cl in range(NCPB):
                normed = ch1p.tile([128, CLOC], BF16, tag="normed")
                nc.vector.tensor_scalar(
                    normed[:], ygat[:, cl * CLOC:(cl + 1) * CLOC],
                    rstd[:, cl:cl + 1], None, ALU.mult)
                nps = psT.tile([128, CLOC], BF16, tag="trans")
                for t in range(NK2):
                    nc.tensor.transpose(
                        nps[:, t * 128:(t + 1) * 128],
                        normed[:, t * 128:(t + 1) * 128], idb_sb[:])
                qdst = qstage[:].rearrange(
                    "p (t s) -> p t s", t=NK2)[:, :, cl * 128:(cl + 1) * 128]
                nsrc = nps[:].rearrange("p (t s) -> p t s", t=NK2)
                nc.scalar.copy(qdst, nsrc)

            while drain:
                emit_outproj(*drain.pop(0))
            pending_out.extend((m, qstage, sb) for m in range(NM2))

        while pending_out:
            emit_outproj(*pending_out.pop(0))


def _emit_chunk(nc, gc, cl, lc, gate_sb, xcs, bcs, ccs,
                csdtT, seg, epb, ygat, ssum,
                idf_sb, idr_sb, idb_sb, trim_sb, dbc_sb,
                st_sb, stT, chp, ch1p, psY, psT, psPB):
    csl = slice(cl * CHUNK, (cl + 1) * CHUNK)

    # gate transpose (silu already applied) -> silg_c
    gps = psT.tile([128, CLOC], BF16, tag="trans")
    for t in range(8):
        nc.tensor.transpose(
            gps[:, t * 128:(t + 1) * 128],
            gate_sb[:, t * SB + cl * CHUNK:t * SB + (cl + 1) * CHUNK],
            idb_sb[:])
    silg = chp.tile([128, CLOC], BF16, tag="silg")
    nc.scalar.copy(silg[:], gps[:])

    # scores = exp(seg) * (triu-in-[s,l] . gram); gram^T = B C^T in [s, l]
    gram_ps = psPB.tile([128, 128], F32, tag="pb", name="gram_ps")
    nc.tensor.matmul(gram_ps[:], bcs[:, csl], ccs[:, csl],
                     start=True, stop=True)
    gram = chp.tile([128, 128], F32, tag="gramm")
    nc.vector.tensor_tensor(gram[:], gram_ps[:], trim_sb[:], ALU.mult)

    # chunk-end decay per head: cend = exp(cs_end), decT = exp(cs_end - cs)
    # (both extracted BEFORE seg/epb are overwritten in place below)
    cend = chp.tile([128, HL], F32, tag="cend")
    ep1 = epb[:, (lc + 1) * CHUNK - 1:(lc + 1) * CHUNK]
    epb_end = bass.AP(tensor=epb.tensor, offset=ep1.offset,
                      ap=[ep1.ap[0], [2 * CHUNK, HL]])
    nc.vector.tensor_copy(cend[:], epb_end)
    decT = chp.tile([128, HL], F32, tag="decT")
    # seg column l=CHUNK-1 holds cs_end - cs[s] (<=0, min-clamp no-op there)
    sg1 = seg[:, CHUNK - 1:CHUNK]
    seg_end = bass.AP(tensor=seg.tensor, offset=sg1.offset,
                      ap=[sg1.ap[0], [CHUNK, HL]])
    nc.scalar.activation(decT[:], seg_end, AF.Exp)

    # scores = exp(seg) * gram -> bf16
    scores = chp.tile([128, HL * CHUNK], BF16, tag="scores", bufs=1)
    nc.scalar.activation(scores[:], seg[:], AF.Exp)
    s3 = scores[:].rearrange("p (h l) -> p h l", h=HL)
    gram_b = bass.AP(tensor=gram.tensor, offset=gram[:].offset,
                     ap=[gram[:].ap[0], [0, HL], [1, 128]])
    nc.vector.tensor_tensor(s3, s3, gram_b, ALU.mult)

    # e4 = exp(pb) * C (for Yoff) -> bf16
    e4 = chp.tile([128, HL * CHUNK], BF16, tag="e4", bufs=1)
    e4_3 = e4[:].rearrange("p (h l) -> p h l", h=HL)
    ep0 = epb[:, lc * CHUNK:lc * CHUNK + 1]
    epb_3 = bass.AP(tensor=epb.tensor, offset=ep0.offset,
                    ap=[ep0.ap[0], [2 * CHUNK, HL], [1, CHUNK]])
    cc0 = ccs[:, cl * CHUNK:cl * CHUNK + 1]
    ccs_b = bass.AP(tensor=ccs.tensor, offset=cc0.offset,
                    ap=[cc0.ap[0], [0, HL], [1, CHUNK]])
    nc.vector.tensor_tensor(e4_3, epb_3, ccs_b, ALU.mult)
    ddt = chp.tile([128, HL], F32, tag="ddt")
    nc.vector.tensor_tensor(ddt[:], csdtT[:, (NCPB + cl) * HL:
                                           (NCPB + cl + 1) * HL],
                            decT[:], ALU.mult)

    # x transpose -> xT (f32), then xdt / xdd
    xT = ch1p.tile([128, CLOC], BF16, tag="xT")
    for hx in range(2):
        xps = psT.tile([128, 512], BF16, tag="trans", name=f"xps{hx}")
        for t in range(4):
            tt = hx * 4 + t
            nc.tensor.transpose(
                xps[:, t * 128:(t + 1) * 128],
                xcs[:, tt * SB + cl * CHUNK:tt * SB + (cl + 1) * CHUNK],
                idb_sb[:])
        nc.scalar.copy(xT[:, hx * 512:(hx + 1) * 512], xps[:])
    xdt = ch1p.tile([128, CLOC], BF16, tag="xdt")
    x3 = xT[:].rearrange("p (h j) -> p h j", h=HL)
    dt0 = csdtT[:, (NCPB + cl) * HL:(NCPB + cl) * HL + 1]
    dt_b = bass.AP(tensor=csdtT.tensor, offset=dt0.offset,
                   ap=[dt0.ap[0], [1, HL], [0, HD]])
    ddt_b = bass.AP(tensor=ddt.tensor, offset=ddt[:].offset,
                    ap=[ddt[:].ap[0], [1, HL], [0, HD]])
    nc.vector.tensor_tensor(xdt[:].rearrange("p (h j) -> p h j", h=HL),
                            x3, dt_b, ALU.mult)
    # ysb = x*D now (before xdd overwrites xT in place)
    dbc_b = bass.AP(tensor=dbc_sb.tensor, offset=dbc_sb[:].offset,
                    ap=[dbc_sb[:].ap[0], [1, HL], [0, HD]])
    ysb = ch1p.tile([128, CLOC], F32, tag="ysb")
    nc.vector.tensor_tensor(ysb[:].rearrange("p (h j) -> p h j", h=HL),
                            x3, dbc_b, ALU.mult)
    xdd = xT  # in place: x * ddt overwrites xT
    nc.vector.tensor_tensor(xdd[:].rearrange("p (h j) -> p h j", h=HL),
                            x3, ddt_b, ALU.mult)

    # B chunk transposed (bln) for state matmuls
    pbt = psPB.tile([128, 128], BF16, tag="pb", name="pbt")
    nc.tensor.transpose(pbt[:], bcs[:, csl], idb_sb[:])
    bln = chp.tile([128, 128], BF16, tag="bln")
    nc.scalar.copy(bln[:], pbt[:])

    # Ydiag + Yoff accumulated per head (two matmuls per head)
    y_halves = []
    for half in range(2):
        y_ps = psY.tile([128, 512], F32, tag="y", name=f"y{half}")
        for hh in range(8):
            h = half * 8 + hh
            hs = slice(hh * HD, (hh + 1) * HD)
            nc.tensor.matmul(
                y_ps[:, hs], scores[:, h * CHUNK:(h + 1) * CHUNK],
                xdt[:, h * HD:(h + 1) * HD], start=True, stop=(gc == 0))
            if gc > 0:
                nc.tensor.matmul(
                    y_ps[:, hs],
                    e4[:, h * CHUNK:(h + 1) * CHUNK],
                    stT[:, h * HD:(h + 1) * HD], start=False, stop=True)
        y_halves.append(y_ps)

    # states for this chunk
    s_halves = []
    for half in range(2):
        s_ps = psY.tile([128, 512], F32, tag="y", name=f"s{half}")
        nc.tensor.matmul(
            s_ps[:], bln[:], xdd[:, half * 512:(half + 1) * 512],
            start=True, stop=True)
        s_halves.append(s_ps)

    # y = (Ydiag + Yoff) + D*x, gated; squares accumulated for RMS
    ysl = slice(cl * CLOC, (cl + 1) * CLOC)
    for half in range(2):
        hsl = slice(half * 512, (half + 1) * 512)
        nc.vector.tensor_tensor(ysb[:, hsl], ysb[:, hsl],
                                y_halves[half][:], ALU.add)
    nc.vector.tensor_tensor(ysb[:], ysb[:], silg[:], ALU.mult)
    nc.scalar.copy(ygat[:, ysl], ysb[:])
    nc.scalar.activation(xdt[:], ysb[:], AF.Square,
                         accum_out=ssum[:, cl:cl + 1])

    # state update: st = st * exp(cs_end) + s  (first chunk: st = s)
    if gc == 0:
        for half in range(2):
            hsl = slice(half * 512, (half + 1) * 512)
            nc.vector.tensor_copy(st_sb[:, hsl], s_halves[half][:])
        nc.vector.tensor_copy(stT[:], st_sb[:])
    else:
        cend_b = bass.AP(tensor=cend.tensor, offset=cend[:].offset,
                         ap=[cend[:].ap[0], [1, HL], [0, HD]])
        st3 = st_sb[:].rearrange("p (h j) -> p h j", h=HL)
        nc.vector.tensor_tensor(st3, st3, cend_b, ALU.mult)
        for half in range(2):
            hsl = slice(half * 512, (half + 1) * 512)
            nc.vector.tensor_tensor(st_sb[:, hsl], st_sb[:, hsl],
                                    s_halves[half][:], ALU.add)
        nc.vector.tensor_copy(stT[:], st_sb[:])


def prepare_in_maps(hidden_states, in_proj_w, conv_w, conv_b, dt_bias, D,
                    norm_w, out_proj_w):
    hidT = np.ascontiguousarray(hidden_states.reshape(S, H_SIZE).T)
    # [half, kk, r, sb, c] -> [sb, half, r, kk, c]
    hids = np.ascontiguousarray(
        hidT.reshape(2, 16, 128, NSB, SB).transpose(3, 0, 2, 1, 4)
        .reshape(NSB, 2, 128, 16 * SB)).astype(NPBF16)
    idf = np.eye(128, dtype=np.float32)
    idb = np.eye(128).astype(NPBF16)
    # mask in [s, l]: keep l >= s
    trim = np.triu(np.ones((128, 128), np.float32)).astype(NPBF16)
    in_maps = []
    for c in range(N_CORES):
        gsl = slice(CLOC * c, CLOC * (c + 1))
        xsl = slice(INTER + CLOC * c, INTER + CLOC * (c + 1))
        bsl = slice(2 * INTER + SS * c, 2 * INTER + SS * (c + 1))
        cslc = slice(2 * INTER + NG * SS + SS * c,
                     2 * INTER + NG * SS + SS * (c + 1))
        dsl = slice(INTER + CONV_DIM + HL * c, INTER + CONV_DIM + HL * (c + 1))
        w1 = np.concatenate([in_proj_w[gsl], in_proj_w[xsl], in_proj_w[bsl],
                             in_proj_w[cslc], in_proj_w[dsl]], axis=0)
        w1 = np.concatenate(
            [w1, np.zeros((NF * 128 - w1.shape[0], H_SIZE), np.float32)],
            axis=0)
        w1f = np.ascontiguousarray(
            w1.T.reshape(2, 16, 128, NF, 128).transpose(3, 0, 2, 1, 4)
            .reshape(NF, 2, 128, 16 * 128)).astype(NPBF16)
        w2 = out_proj_w[:, gsl] * norm_w[gsl][None, :]  # norm_w folded
        w2m = np.ascontiguousarray(
            w2.T.reshape(NK2, 128, NM2, 128).transpose(2, 1, 0, 3)
            .reshape(NM2 // 4, 4 * 128 * NK2 * 128 // 1024, 1024)
            .reshape(NM2 // 4, 4, 128, NK2 * 128).transpose(0, 2, 1, 3)
            .reshape(NM2 // 4, 128, 4 * NK2 * 128)).astype(NPBF16)
        conv_idx = np.concatenate([
            np.arange(CLOC * c, CLOC * (c + 1)),
            np.arange(INTER + SS * c, INTER + SS * (c + 1)),
            np.arange(INTER + NG * SS + SS * c,
                      INTER + NG * SS + SS * (c + 1))])
        cwl = conv_w[conv_idx, 0, :]          # [1280, 4]
        cbl = conv_b[conv_idx]                # [1280]
        convw = np.ascontiguousarray(
            cwl.reshape(10, 128, KCONV).transpose(1, 0, 2)
            .reshape(128, 10 * KCONV))
        convb = np.ascontiguousarray(cbl.reshape(10, 128).transpose(1, 0))
        hsl = slice(HL * c, HL * (c + 1))
        acol = -(np.arange(HL * c + 1, HL * (c + 1) + 1, dtype=np.float32))
        in_maps.append({
            "hids": hids,
            "w1f": w1f,
            "w2m": w2m,
            "convw": convw,
            "convb": convb,
            "dtbias": dt_bias[hsl].reshape(HL, 1).astype(np.float32),
            "acol": acol.reshape(HL, 1),
            "dbc": np.tile(D[hsl][None, :], (128, 1)).astype(np.float32),
            "idf": idf,
            "idb": idb,
            "trim": trim,
        })
    return in_maps


def get_nc():
    if "nc" not in _CACHE:
        _CACHE["nc"] = build_nc()
    return _CACHE["nc"]


def kernel(hidden_states, in_proj_w, conv_w, conv_b, dt_bias, D, norm_w,
           out_proj_w):
    nc = get_nc()
    in_maps = prepare_in_maps(
        np.asarray(hidden_states, np.float32),
        np.asarray(in_proj_w, np.float32),
        np.asarray(conv_w, np.float32), np.asarray(conv_b, np.float32),
        np.asarray(dt_bias, np.float32), np.asarray(D, np.float32),
        np.asarray(norm_w, np.float32), np.asarray(out_proj_w, np.float32))
    res = run_bass_kernel_spmd(nc, in_maps, list(range(N_CORES)))
    acc = np.zeros((H_SIZE, S), np.float64)
    for r in res.results:
        acc += np.asarray(r["outp"], np.float64).transpose(0, 2, 1, 3) \
                 .reshape(H_SIZE, S)
    return acc.T.astype(np.float32).reshape(1, S, H_SIZE)


# revision 29
# speedup vs baseline: 1.8198x; 1.0179x over previous
"""NemotronH Mamba2 mixer on 8 Trainium2 cores (Bass/Tile).

Sharding: tensor-parallel over heads/groups. Core c owns group c =
16 heads (= 1024 gate/x channels, 128 B + 128 C state channels, 16 dt).
in_proj rows and out_proj columns are sharded accordingly; out_proj is
row-parallel over the contraction, partials are combined on the host.

v2: bf16 matmul operands everywhere (fp32 kept on the decay/cumsum,
softplus and state-accumulation paths), per-head cs broadcast via
full-rate fp32r matmuls (free dim 256 over chunk pairs), softplus and
rsqrt via the exp+ln activation table (two table loads per superblock),
conv taps on GPSIMD, per-head scalars applied with stride-0 3D
broadcast APs in single wide DVE ops, bf16 output partials.
"""

import numpy as np
import ml_dtypes

import concourse.bass as bass
import concourse.mybir as mybir
from concourse import bacc
from concourse.tile import TileContext
from concourse.bass_utils import run_bass_kernel_spmd

F32 = mybir.dt.float32
F32R = mybir.dt.float32r
BF16 = mybir.dt.bfloat16
AF = mybir.ActivationFunctionType
ALU = mybir.AluOpType
NPBF16 = ml_dtypes.bfloat16

# Model dims
H_SIZE = 4096
NH = 128
HD = 64
SS = 128
KCONV = 4
NG = 8
CHUNK = 128
INTER = NH * HD                 # 8192
CONV_DIM = INTER + 2 * NG * SS  # 10240
PROJ = INTER + CONV_DIM + NH    # 18560
DT_MIN, DT_MAX = 0.001, 100.0
EPS = 1e-5
GROUP = INTER // NG             # 1024

# Sharding / tiling
N_CORES = 8
S = 2048
HL = NH // N_CORES              # 16 local heads
CLOC = HL * HD                  # 1024 local gate/x channels
NSB = 4                         # seq superblocks
SB = S // NSB                   # 512
NCPB = SB // CHUNK              # 4 chunks per superblock
NF = 19                         # f-tiles: 8 gate + 8 x + B + C + dt
NK1 = H_SIZE // 128             # 32 k-tiles for in_proj
NK2 = CLOC // 128               # 8 k-tiles for out_proj
NM2 = H_SIZE // 128             # 32 m-tiles for out_proj

_CACHE = {}


def r32(ap):
    return ap.bitcast(F32R)


def build_nc():
    nc = bacc.Bacc(None, target_bir_lowering=False)

    # hidden, pre-tiled: [sb, half, 128, 16*512] bf16 (chan-major k-tiles)
    hids = nc.declare_dram_parameter("hids", [NSB, 2, 128, 16 * SB], BF16,
                                     isOutput=False)
    # in_proj weights per f-tile: [f, half, 128, 16*128] bf16
    w1f = nc.declare_dram_parameter("w1f", [NF, 2, 128, 16 * 128], BF16,
                                    isOutput=False)
    # out_proj weights, groups of 4 m-tiles: [m/4, 128, 4*8*128] bf16
    w2m = nc.declare_dram_parameter("w2m", [NM2 // 4, 128, 4 * NK2 * 128],
                                    BF16, isOutput=False)
    convw = nc.declare_dram_parameter("convw", [128, 10 * KCONV], F32,
                                      isOutput=False)
    convb = nc.declare_dram_parameter("convb", [128, 10], F32, isOutput=False)
    dtbias = nc.declare_dram_parameter("dtbias", [HL, 1], F32, isOutput=False)
    acol = nc.declare_dram_parameter("acol", [HL, 1], F32, isOutput=False)
    dbc = nc.declare_dram_parameter("dbc", [128, HL], F32, isOutput=False)
    idf = nc.declare_dram_parameter("idf", [128, 128], F32, isOutput=False)
    idb = nc.declare_dram_parameter("idb", [128, 128], BF16, isOutput=False)
    trim = nc.declare_dram_parameter("trim", [128, 128], BF16, isOutput=False)
    outp = nc.declare_dram_parameter("outp", [NM2, NSB, 128, SB], BF16,
                                     isOutput=True)

    with TileContext(nc) as tc:
        with tc.tile_pool(name="const", bufs=1) as cp:
            idf_sb = cp.tile([128, 128], F32, tag="idf")
            idr_sb = cp.tile([128, 128], F32R, tag="idr")
            idb_sb = cp.tile([128, 128], BF16, tag="idb")
            trim_sb = cp.tile([128, 128], BF16, tag="trim")
            cw_sb = cp.tile([128, 10 * KCONV], F32, tag="cw")
            cb_sb = cp.tile([128, 10], F32, tag="cb")
            dtb_sb = cp.tile([HL, 1], F32, tag="dtb")
            a_sb = cp.tile([HL, 1], F32, tag="acol")
            dbc_sb = cp.tile([128, HL], F32, tag="dbc")
            ones16 = cp.tile([HL, 1], F32, tag="ones16")
            zcol = cp.tile([128, 1], F32, tag="zcol")
            st_sb = cp.tile([128, HL * HD], F32R, tag="state")
            stT = cp.tile([128, HL * HD], BF16, tag="stateb")
            nc.sync.dma_start(out=idf_sb[:], in_=idf[:])
            nc.sync.dma_start(out=idr_sb[:], in_=r32(idf[:]))
            nc.sync.dma_start(out=idb_sb[:], in_=idb[:])
            nc.sync.dma_start(out=trim_sb[:], in_=trim[:])
            nc.sync.dma_start(out=cw_sb[:], in_=convw[:])
            nc.sync.dma_start(out=cb_sb[:], in_=convb[:])
            nc.sync.dma_start(out=dtb_sb[:], in_=dtbias[:])
            nc.sync.dma_start(out=a_sb[:], in_=acol[:])
            nc.sync.dma_start(out=dbc_sb[:], in_=dbc[:])
            nc.vector.memset(ones16[:], 1.0)
            nc.vector.memset(zcol[:], 0.0)

            _main_phase(nc, tc, hids, w1f, w2m, outp,
                        idf_sb, idr_sb, idb_sb, trim_sb, cw_sb, cb_sb,
                        dtb_sb, a_sb, dbc_sb, ones16, zcol, st_sb, stT)

    nc.compile()
    return nc


def _main_phase(nc, tc, hids, w1f, w2m, outp,
                idf_sb, idr_sb, idb_sb, trim_sb, cw_sb, cb_sb,
                dtb_sb, a_sb, dbc_sb, ones16, zcol, st_sb, stT):
    with tc.tile_pool(name="hid", bufs=1) as hidp, \
         tc.tile_pool(name="w1", bufs=3) as w1p, \
         tc.tile_pool(name="w2", bufs=2) as w2p, \
         tc.tile_pool(name="stage", bufs=1) as sgp, \
         tc.tile_pool(name="qst", bufs=2) as qstp, \
         tc.tile_pool(name="conv32", bufs=2) as cvp, \
         tc.tile_pool(name="pair", bufs=1) as prp, \
         tc.tile_pool(name="seg", bufs=2) as segp, \
         tc.tile_pool(name="ch", bufs=2) as chp, \
         tc.tile_pool(name="ch1", bufs=1) as ch1p, \
         tc.tile_pool(name="oev", bufs=2) as oevp, \
         tc.tile_pool(name="acc", bufs=2, space="PSUM") as accp, \
         tc.tile_pool(name="psY", bufs=2, space="PSUM") as psY, \
         tc.tile_pool(name="psPB", bufs=2, space="PSUM") as psPB, \
         tc.tile_pool(name="psT", bufs=2, space="PSUM") as psT:

        # conv input staging: 10 channel tiles (8 x, 1 B, 1 C), 3 halo + SB
        ccat = sgp.tile([128, 10 * (SB + 3)], F32, tag="ccat")
        for t in range(10):
            nc.vector.memset(ccat[:, t * (SB + 3):t * (SB + 3) + 3], 0.0)

        pending_out = []

        w2cache = {}

        def load_w2(G):
            if w2cache.get("G") == G:
                return w2cache["t"]
            w2 = w2p.tile([128, 4 * NK2 * 128], BF16, tag="w2")
            nc.sync.dma_start(out=w2[:], in_=w2m[G])
            w2cache["G"] = G
            w2cache["t"] = w2
            w2cache.setdefault("loaded", set()).add(G)
            return w2

        def emit_outproj(m, qst, sbq):
            G, g = m // 4, m % 4
            if w2cache.get("G") != G:
                # use prefetched tile if the ring already holds it
                w2 = w2cache.get("pref") if w2cache.get("prefG") == G else None
                if w2 is None:
                    w2 = w2p.tile([128, 4 * NK2 * 128], BF16, tag="w2")
                    nc.sync.dma_start(out=w2[:], in_=w2m[G])
                w2cache["G"] = G
                w2cache["t"] = w2
                w2cache["pref"] = None
            w2 = w2cache["t"]
            if g == 0 and G + 1 < NM2 // 4 and w2cache.get("prefG") != G + 1:
                pw = w2p.tile([128, 4 * NK2 * 128], BF16, tag="w2",
                              name="w2pref")
                nc.sync.dma_start(out=pw[:], in_=w2m[G + 1])
                w2cache["prefG"] = G + 1
                w2cache["pref"] = pw
            acc = accp.tile([128, SB], F32, tag="acc")
            for kt in range(NK2):
                nc.tensor.matmul(
                    acc[:], w2[:, (g * NK2 + kt) * 128:(g * NK2 + kt + 1) * 128],
                    qst[:, kt * SB:kt * SB + SB],
                    start=(kt == 0), stop=(kt == NK2 - 1))
            ev = oevp.tile([128, SB], BF16, tag="oev")
            nc.scalar.copy(ev[:], acc[:])
            nc.sync.dma_start(out=outp[m, sbq], in_=ev[:])

        for sb in range(NSB):
            w1pre = []
            for khalf in range(2):
                w1t_ = w1p.tile([128, 16 * 128], BF16, tag="w1",
                                name=f"w1pre{khalf}")
                nc.sync.dma_start(out=w1t_[:], in_=w1f[18, khalf])
                w1pre.append(w1t_)
            quarters = []
            for kq in range(8):
                hid_q = hidp.tile([128, 4 * SB], BF16, tag=f"hid{kq}",
                                  name=f"hid{kq}")
                nc.sync.dma_start(
                    out=hid_q[:],
                    in_=hids[sb, kq // 4][:, (kq % 4) * 4 * SB:
                                          (kq % 4 + 1) * 4 * SB])
                quarters.append(hid_q)

            gate_sb = sgp.tile([128, 8 * SB], BF16, tag="gate")
            dtraw = sgp.tile([HL, SB], F32, tag="dtraw")

            # halo copies must read previous superblock before overwrite
            if sb > 0:
                for t in range(10):
                    base = t * (SB + 3)
                    nc.vector.tensor_copy(
                        ccat[:, base:base + 3], ccat[:, base + SB:base + SB + 3])

            def emit_ftile(f, gate_sb=gate_sb, dtraw=dtraw,
                           quarters=quarters, w1pre=w1pre):
                if f == 18:
                    w1h = w1pre
                else:
                    w1h = []
                    for khalf in range(2):
                        w1t_ = w1p.tile([128, 16 * 128], BF16, tag="w1")
                        nc.sync.dma_start(out=w1t_[:], in_=w1f[f, khalf])
                        w1h.append(w1t_)
                acc = accp.tile([128, SB], F32, tag="acc")
                for k in range(NK1):
                    nc.tensor.matmul(
                        acc[:],
                        w1h[k // 16][:, (k % 16) * 128:(k % 16 + 1) * 128],
                        quarters[k // 4][:, (k % 4) * SB:(k % 4 + 1) * SB],
                        start=(k == 0), stop=(k == NK1 - 1))
                if f < 8:
                    nc.scalar.copy(gate_sb[:, f * SB:(f + 1) * SB], acc[:])
                elif f < 18:
                    t = f - 8
                    base = t * (SB + 3)
                    nc.scalar.copy(ccat[:, base + 3:base + 3 + SB], acc[:])
                else:
                    nc.scalar.copy(dtraw[:, :], acc[:HL, :])

            drain = list(pending_out)
            pending_out.clear()
            for u in [18, 16, 17] + list(range(8, 16)) + list(range(8)):
                emit_ftile(u)

            # ---------------- dt pipeline (exp/ln softplus) ----------------
            # z = dtraw + dt_bias; sp = relu(z) + ln(1 + exp(-|z|))
            az = sgp.tile([HL, SB], F32, tag="az")
            dtsp = dtraw  # in-place: relu(z)+ln1p overwrites raw dt
            cs = sgp.tile([HL, SB], F32, tag="cs")
            nc.scalar.activation(az[:], dtraw[:], AF.Abs, bias=dtb_sb[:, 0:1])
            nc.scalar.activation(az[:], az[:], AF.Exp, scale=-1.0)
            nc.vector.tensor_scalar(az[:], az[:], 1.0, None, ALU.add)
            nc.scalar.activation(az[:], az[:], AF.Ln)
            nc.scalar.activation(dtsp[:], dtraw[:], AF.Relu,
                                 bias=dtb_sb[:, 0:1])
            nc.vector.tensor_tensor(dtsp[:], dtsp[:], az[:], ALU.add)
            nc.vector.tensor_scalar(dtsp[:], dtsp[:], DT_MIN, DT_MAX,
                                    ALU.max, ALU.min)
            dA = az  # az dead, reuse
            nc.vector.tensor_scalar(dA[:], dtsp[:], a_sb[:, 0:1], None,
                                    ALU.mult)
            for cl in range(NCPB):
                ones_b = bass.AP(tensor=ones16.tensor,
                                 offset=ones16[:].offset,
                                 ap=[ones16[:].ap[0], [0, CHUNK]])
                nc.vector.tensor_tensor_scan(
                    cs[:, cl * CHUNK:(cl + 1) * CHUNK],
                    ones_b, dA[:, cl * CHUNK:(cl + 1) * CHUNK],
                    0.0, ALU.mult, ALU.add)

            # SiLU on gate (batched with conv SiLUs: one table load)
            nc.scalar.activation(gate_sb[:], gate_sb[:], AF.Silu)

            # ---------------- conv taps (GPSIMD) + SiLU ----------------
            xcs = sgp.tile([128, 8 * SB], BF16, tag="xcs")
            bcs = sgp.tile([128, SB], BF16, tag="bcs")
            ccs = sgp.tile([128, SB], BF16, tag="ccs")
            for t in [8, 9] + list(range(8)):
                base = t * (SB + 3)
                eng = nc.vector
                c32 = cvp.tile([128, SB], F32, tag="c32")
                eng.tensor_scalar(
                    c32[:], ccat[:, base:base + SB],
                    cw_sb[:, t * KCONV:t * KCONV + 1], cb_sb[:, t:t + 1],
                    ALU.mult, ALU.add)
                for j in range(1, KCONV):
                    eng.scalar_tensor_tensor(
                        c32[:], ccat[:, base + j:base + j + SB],
                        cw_sb[:, t * KCONV + j:t * KCONV + j + 1], c32[:],
                        ALU.mult, ALU.add)
                dst = (xcs[:, t * SB:(t + 1) * SB] if t < 8
                       else (bcs[:] if t == 8 else ccs[:]))
                nc.scalar.activation(dst, c32[:], AF.Silu)

            # csT/dtT for all chunks: [128, cl*HL + h] / [128, (4+cl)*HL + h]
            pcs = psPB.tile([128, 2 * NCPB * HL], F32, tag="pb",
                            name="pcs")
            for cl in range(NCPB):
                nc.tensor.transpose(
                    pcs[:, cl * HL:(cl + 1) * HL],
                    cs[:, cl * CHUNK:(cl + 1) * CHUNK], idf_sb[:HL, :HL])
                nc.tensor.transpose(
                    pcs[:, (NCPB + cl) * HL:(NCPB + cl + 1) * HL],
                    dtsp[:, cl * CHUNK:(cl + 1) * CHUNK], idf_sb[:HL, :HL])
            csdtT = sgp.tile([128, 2 * NCPB * HL], F32, tag="csdtT")
            nc.scalar.copy(csdtT[:], pcs[:])
            negcsT = sgp.tile([128, NCPB * HL], F32, tag="negcsT")
            nc.vector.tensor_scalar(negcsT[:], csdtT[:, :NCPB * HL], -1.0,
                                    None, ALU.mult)

            qstage = qstp.tile([128, NK2 * SB], BF16, tag="qstage")
            ygat = sgp.tile([128, NCPB * CLOC], BF16, tag="ygat")
            ssum = sgp.tile([128, NCPB], F32, tag="ssum")

            # ---------------- SSD chunk pairs ----------------
            for pr in range(NCPB // 2):
                prsl = slice(pr * 2 * CHUNK, (pr + 1) * 2 * CHUNK)
                # exact-ish bf16 triple split of cs for full-rate broadcasts
                csh = sgp.tile([HL, 2 * CHUNK], BF16, tag="csh", bufs=1,
                               name="csh")
                csm = sgp.tile([HL, 2 * CHUNK], BF16, tag="csm", bufs=1,
                               name="csm")
                csl_ = sgp.tile([HL, 2 * CHUNK], BF16, tag="csl", bufs=1,
                               name="csl")
                res = sgp.tile([HL, 2 * CHUNK], F32, tag="csres", bufs=1,
                               name="res")
                nc.vector.tensor_copy(csh[:], cs[:, prsl])
                nc.vector.tensor_tensor(res[:], cs[:, prsl], csh[:],
                                        ALU.subtract)
                nc.vector.tensor_copy(csm[:], res[:])
                nc.vector.tensor_tensor(res[:], res[:], csm[:], ALU.subtract)
                nc.vector.tensor_copy(csl_[:], res[:])
                # per-head cs broadcast: pb[p, j*256+l] = cs[h, pr*256+l]
                # (fp32r, free 256 -> full rate), 2 heads per PSUM tile
                epb = prp.tile([128, HL * 2 * CHUNK], F32R, tag="epb")
                segs = [segp.tile([128, HL * CHUNK], F32R, tag="seg",
                                  name=f"seg{i}")
                        for i in range(2)]
                for hg in range(HL // 2):
                    pb = psPB.tile([128, 512], F32, tag="pb")
                    for j in range(2):
                        h = 2 * hg + j
                        idcol = idb_sb[:HL, h:h + 1]
                        indh = bass.AP(tensor=idcol.tensor,
                                       offset=idcol.offset,
                                       ap=[[idcol.ap[0][0], HL], [0, 128]])
                        for si, spl in enumerate((csh, csm, csl_)):
                            nc.tensor.matmul(pb[:, j * 256:(j + 1) * 256],
                                             indh, spl[:],
                                             start=(si == 0), stop=(si == 2))
                    # seg[s, l] = min(cs[h,l] - cs[h,s], 0) per chunk
                    for lc in range(2):
                        cl = 2 * pr + lc
                        for j in range(2):
                            h = 2 * hg + j
                            nc.vector.scalar_tensor_tensor(
                                segs[lc][:, h * CHUNK:(h + 1) * CHUNK],
                                pb[:, j * 256 + lc * 128:
                                   j * 256 + (lc + 1) * 128],
                                negcsT[:, cl * HL + h:cl * HL + h + 1],
                                bass.AP(tensor=zcol.tensor,
                                        offset=zcol[:].offset,
                                        ap=[zcol[:].ap[0], [0, CHUNK]]),
                                ALU.add, ALU.min)
                    nc.scalar.activation(epb[:, hg * 512:(hg + 1) * 512],
                                         pb[:], AF.Exp)

                for lc in range(2):
                    cl = 2 * pr + lc
                    def filler(n, drain=drain):
                        for _ in range(min(n, len(drain))):
                            emit_outproj(*drain.pop(0))
                    _emit_chunk(nc, sb * NCPB + cl, cl, lc, gate_sb, xcs,
                                bcs, ccs,
                                csdtT, segs[lc], epb, ygat, ssum,
                                idf_sb, idr_sb, idb_sb, trim_sb, dbc_sb,
                                st_sb, stT, chp, ch1p, psY, psT, psPB,
                                filler)

            # ---------------- deferred group RMSNorm + transposes ----------
            lnm = chp.tile([128, NCPB], F32, tag="lnm")
            rstd = chp.tile([128, NCPB], F32, tag="rstd")
            nc.vector.tensor_scalar(lnm[:], ssum[:], 1.0 / GROUP, EPS,
                                    ALU.mult, ALU.add)
            nc.scalar.activation(lnm[:], lnm[:], AF.Ln)
            nc.scalar.activation(rstd[:], lnm[:], AF.Exp, scale=-0.5)
            for cl in range(NCPB):
                normed = ch1p.tile([128, CLOC], BF16, tag="normed")
                nc.vector.tensor_scalar(
                    normed[:], ygat[:, cl * CLOC:(cl + 1) * CLOC],
                    rstd[:, cl:cl + 1], None, ALU.mult)
                nps = psT.tile([128, CLOC], BF16, tag="trans")
                for t in range(NK2):
                    nc.tensor.transpose(
                        nps[:, t * 128:(t + 1) * 128],
                        normed[:, t * 128:(t + 1) * 128], idb_sb[:])
                qdst = qstage[:].rearrange(
                    "p (t s) -> p t s", t=NK2)[:, :, cl * 128:(cl + 1) * 128]
                nsrc = nps[:].rearrange("p (t s) -> p t s", t=NK2)
                nc.scalar.copy(qdst, nsrc)

            while drain:
                emit_outproj(*drain.pop(0))
            pending_out.extend((m, qstage, sb) for m in range(NM2))

        while pending_out:
            emit_outproj(*pending_out.pop(0))


def _emit_chunk(nc, gc, cl, lc, gate_sb, xcs, bcs, ccs,
                csdtT, seg, epb, ygat, ssum,
                idf_sb, idr_sb, idb_sb, trim_sb, dbc_sb,
                st_sb, stT, chp, ch1p, psY, psT, psPB, filler):
    csl = slice(cl * CHUNK, (cl + 1) * CHUNK)

    # gate transpose (silu already applied) -> silg_c
    gps = psT.tile([128, CLOC], BF16, tag="trans")
    for t in range(8):
        nc.tensor.transpose(
            gps[:, t * 128:(t + 1) * 128],
            gate_sb[:, t * SB + cl * CHUNK:t * SB + (cl + 1) * CHUNK],
            idb_sb[:])
    silg = chp.tile([128, CLOC], BF16, tag="silg")
    nc.scalar.copy(silg[:], gps[:])

    # scores = exp(seg) * (triu-in-[s,l] . gram); gram^T = B C^T in [s, l]
    gram_ps = psPB.tile([128, 128], F32, tag="pb", name="gram_ps")
    nc.tensor.matmul(gram_ps[:], bcs[:, csl], ccs[:, csl],
                     start=True, stop=True)
    gram = chp.tile([128, 128], F32, tag="gramm")
    nc.vector.tensor_tensor(gram[:], gram_ps[:], trim_sb[:], ALU.mult)

    # chunk-end decay per head: cend = exp(cs_end), decT = exp(cs_end - cs)
    # (both extracted BEFORE seg/epb are overwritten in place below)
    cend = chp.tile([128, HL], F32, tag="cend")
    ep1 = epb[:, (lc + 1) * CHUNK - 1:(lc + 1) * CHUNK]
    epb_end = bass.AP(tensor=epb.tensor, offset=ep1.offset,
                      ap=[ep1.ap[0], [2 * CHUNK, HL]])
    nc.vector.tensor_copy(cend[:], epb_end)
    decT = chp.tile([128, HL], F32, tag="decT")
    # seg column l=CHUNK-1 holds cs_end - cs[s] (<=0, min-clamp no-op there)
    sg1 = seg[:, CHUNK - 1:CHUNK]
    seg_end = bass.AP(tensor=seg.tensor, offset=sg1.offset,
                      ap=[sg1.ap[0], [CHUNK, HL]])
    nc.scalar.activation(decT[:], seg_end, AF.Exp)

    # scores = exp(seg) * gram -> bf16
    scores = chp.tile([128, HL * CHUNK], BF16, tag="scores", bufs=1)
    nc.scalar.activation(scores[:], seg[:], AF.Exp)
    s3 = scores[:].rearrange("p (h l) -> p h l", h=HL)
    gram_b = bass.AP(tensor=gram.tensor, offset=gram[:].offset,
                     ap=[gram[:].ap[0], [0, HL], [1, 128]])
    nc.vector.tensor_tensor(s3, s3, gram_b, ALU.mult)

    # e4 = exp(pb) * C (for Yoff) -> bf16
    e4 = chp.tile([128, HL * CHUNK], BF16, tag="e4", bufs=1)
    e4_3 = e4[:].rearrange("p (h l) -> p h l", h=HL)
    ep0 = epb[:, lc * CHUNK:lc * CHUNK + 1]
    epb_3 = bass.AP(tensor=epb.tensor, offset=ep0.offset,
                    ap=[ep0.ap[0], [2 * CHUNK, HL], [1, CHUNK]])
    cc0 = ccs[:, cl * CHUNK:cl * CHUNK + 1]
    ccs_b = bass.AP(tensor=ccs.tensor, offset=cc0.offset,
                    ap=[cc0.ap[0], [0, HL], [1, CHUNK]])
    nc.vector.tensor_tensor(e4_3, epb_3, ccs_b, ALU.mult)
    ddt = chp.tile([128, HL], F32, tag="ddt")
    nc.vector.tensor_tensor(ddt[:], csdtT[:, (NCPB + cl) * HL:
                                           (NCPB + cl + 1) * HL],
                            decT[:], ALU.mult)

    # x transpose -> xT (f32), then xdt / xdd
    xT = ch1p.tile([128, CLOC], BF16, tag="xT")
    for hx in range(2):
        xps = psT.tile([128, 512], BF16, tag="trans", name=f"xps{hx}")
        for t in range(4):
            tt = hx * 4 + t
            nc.tensor.transpose(
                xps[:, t * 128:(t + 1) * 128],
                xcs[:, tt * SB + cl * CHUNK:tt * SB + (cl + 1) * CHUNK],
                idb_sb[:])
        nc.scalar.copy(xT[:, hx * 512:(hx + 1) * 512], xps[:])
    xdt = ch1p.tile([128, CLOC], BF16, tag="xdt")
    x3 = xT[:].rearrange("p (h j) -> p h j", h=HL)
    dt0 = csdtT[:, (NCPB + cl) * HL:(NCPB + cl) * HL + 1]
    dt_b = bass.AP(tensor=csdtT.tensor, offset=dt0.offset,
                   ap=[dt0.ap[0], [1, HL], [0, HD]])
    ddt_b = bass.AP(tensor=ddt.tensor, offset=ddt[:].offset,
                    ap=[ddt[:].ap[0], [1, HL], [0, HD]])
    nc.vector.tensor_tensor(xdt[:].rearrange("p (h j) -> p h j", h=HL),
                            x3, dt_b, ALU.mult)
    # ysb = x*D now (before xdd overwrites xT in place)
    dbc_b = bass.AP(tensor=dbc_sb.tensor, offset=dbc_sb[:].offset,
                    ap=[dbc_sb[:].ap[0], [1, HL], [0, HD]])
    ysb = ch1p.tile([128, CLOC], F32, tag="ysb")
    nc.vector.tensor_tensor(ysb[:].rearrange("p (h j) -> p h j", h=HL),
                            x3, dbc_b, ALU.mult)
    xdd = xT  # in place: x * ddt overwrites xT
    nc.vector.tensor_tensor(xdd[:].rearrange("p (h j) -> p h j", h=HL),
                            x3, ddt_b, ALU.mult)

    # B chunk transposed (bln) for state matmuls
    pbt = psPB.tile([128, 128], BF16, tag="pb", name="pbt")
    nc.tensor.transpose(pbt[:], bcs[:, csl], idb_sb[:])
    bln = chp.tile([128, 128], BF16, tag="bln")
    nc.scalar.copy(bln[:], pbt[:])

    filler(3)

    # Ydiag + Yoff accumulated per head (two matmuls per head)
    y_halves = []
    for half in range(2):
        y_ps = psY.tile([128, 512], F32, tag="y", name=f"y{half}")
        for hh in range(8):
            h = half * 8 + hh
            hs = slice(hh * HD, (hh + 1) * HD)
            nc.tensor.matmul(
                y_ps[:, hs], scores[:, h * CHUNK:(h + 1) * CHUNK],
                xdt[:, h * HD:(h + 1) * HD], start=True, stop=(gc == 0))
            if gc > 0:
                nc.tensor.matmul(
                    y_ps[:, hs],
                    e4[:, h * CHUNK:(h + 1) * CHUNK],
                    stT[:, h * HD:(h + 1) * HD], start=False, stop=True)
        y_halves.append(y_ps)

    # states for this chunk
    s_halves = []
    for half in range(2):
        s_ps = psY.tile([128, 512], F32, tag="y", name=f"s{half}")
        nc.tensor.matmul(
            s_ps[:], bln[:], xdd[:, half * 512:(half + 1) * 512],
            start=True, stop=True)
        s_halves.append(s_ps)

    filler(5)

    # y = (Ydiag + Yoff) + D*x, gated; squares accumulated for RMS
    ysl = slice(cl * CLOC, (cl + 1) * CLOC)
    for half in range(2):
        hsl = slice(half * 512, (half + 1) * 512)
        nc.vector.tensor_tensor(ysb[:, hsl], ysb[:, hsl],
                                y_halves[half][:], ALU.add)
    nc.vector.tensor_tensor(ysb[:], ysb[:], silg[:], ALU.mult)
    nc.scalar.copy(ygat[:, ysl], ysb[:])
    nc.scalar.activation(xdt[:], ysb[:], AF.Square,
                         accum_out=ssum[:, cl:cl + 1])

    # state update: st = st * exp(cs_end) + s  (first chunk: st = s)
    if gc == 0:
        for half in range(2):
            hsl = slice(half * 512, (half + 1) * 512)
            nc.vector.tensor_copy(st_sb[:, hsl], s_halves[half][:])
        nc.vector.tensor_copy(stT[:], st_sb[:])
    else:
        cend_b = bass.AP(tensor=cend.tensor, offset=cend[:].offset,
                         ap=[cend[:].ap[0], [1, HL], [0, HD]])
        st3 = st_sb[:].rearrange("p (h j) -> p h j", h=HL)
        nc.vector.tensor_tensor(st3, st3, cend_b, ALU.mult)
        for half in range(2):
            hsl = slice(half * 512, (half + 1) * 512)
            nc.vector.tensor_tensor(st_sb[:, hsl], st_sb[:, hsl],
                                    s_halves[half][:], ALU.add)
        nc.vector.tensor_copy(stT[:], st_sb[:])


def prepare_in_maps(hidden_states, in_proj_w, conv_w, conv_b, dt_bias, D,
                    norm_w, out_proj_w):
    hidT = np.ascontiguousarray(hidden_states.reshape(S, H_SIZE).T)
    # [half, kk, r, sb, c] -> [sb, half, r, kk, c]
    hids = np.ascontiguousarray(
        hidT.reshape(2, 16, 128, NSB, SB).transpose(3, 0, 2, 1, 4)
        .reshape(NSB, 2, 128, 16 * SB)).astype(NPBF16)
    idf = np.eye(128, dtype=np.float32)
    idb = np.eye(128).astype(NPBF16)
    # mask in [s, l]: keep l >= s
    trim = np.triu(np.ones((128, 128), np.float32)).astype(NPBF16)
    in_maps = []
    for c in range(N_CORES):
        gsl = slice(CLOC * c, CLOC * (c + 1))
        xsl = slice(INTER + CLOC * c, INTER + CLOC * (c + 1))
        bsl = slice(2 * INTER + SS * c, 2 * INTER + SS * (c + 1))
        cslc = slice(2 * INTER + NG * SS + SS * c,
                     2 * INTER + NG * SS + SS * (c + 1))
        dsl = slice(INTER + CONV_DIM + HL * c, INTER + CONV_DIM + HL * (c + 1))
        w1 = np.concatenate([in_proj_w[gsl], in_proj_w[xsl], in_proj_w[bsl],
                             in_proj_w[cslc], in_proj_w[dsl]], axis=0)
        w1 = np.concatenate(
            [w1, np.zeros((NF * 128 - w1.shape[0], H_SIZE), np.float32)],
            axis=0)
        w1f = np.ascontiguousarray(
            w1.T.reshape(2, 16, 128, NF, 128).transpose(3, 0, 2, 1, 4)
            .reshape(NF, 2, 128, 16 * 128)).astype(NPBF16)
        w2 = out_proj_w[:, gsl] * norm_w[gsl][None, :]  # norm_w folded
        w2m = np.ascontiguousarray(
            w2.T.reshape(NK2, 128, NM2, 128).transpose(2, 1, 0, 3)
            .reshape(NM2 // 4, 4 * 128 * NK2 * 128 // 1024, 1024)
            .reshape(NM2 // 4, 4, 128, NK2 * 128).transpose(0, 2, 1, 3)
            .reshape(NM2 // 4, 128, 4 * NK2 * 128)).astype(NPBF16)
        conv_idx = np.concatenate([
            np.arange(CLOC * c, CLOC * (c + 1)),
            np.arange(INTER + SS * c, INTER + SS * (c + 1)),
            np.arange(INTER + NG * SS + SS * c,
                      INTER + NG * SS + SS * (c + 1))])
        cwl = conv_w[conv_idx, 0, :]          # [1280, 4]
        cbl = conv_b[conv_idx]                # [1280]
        convw = np.ascontiguousarray(
            cwl.reshape(10, 128, KCONV).transpose(1, 0, 2)
            .reshape(128, 10 * KCONV))
        convb = np.ascontiguousarray(cbl.reshape(10, 128).transpose(1, 0))
        hsl = slice(HL * c, HL * (c + 1))
        acol = -(np.arange(HL * c + 1, HL * (c + 1) + 1, dtype=np.float32))
        in_maps.append({
            "hids": hids,
            "w1f": w1f,
            "w2m": w2m,
            "convw": convw,
            "convb": convb,
            "dtbias": dt_bias[hsl].reshape(HL, 1).astype(np.float32),
            "acol": acol.reshape(HL, 1),
            "dbc": np.tile(D[hsl][None, :], (128, 1)).astype(np.float32),
            "idf": idf,
            "idb": idb,
            "trim": trim,
        })
    return in_maps


def get_nc():
    if "nc" not in _CACHE:
        _CACHE["nc"] = build_nc()
    return _CACHE["nc"]


def kernel(hidden_states, in_proj_w, conv_w, conv_b, dt_bias, D, norm_w,
           out_proj_w):
    nc = get_nc()
    in_maps = prepare_in_maps(
        np.asarray(hidden_states, np.float32),
        np.asarray(in_proj_w, np.float32),
        np.asarray(conv_w, np.float32), np.asarray(conv_b, np.float32),
        np.asarray(dt_bias, np.float32), np.asarray(D, np.float32),
        np.asarray(norm_w, np.float32), np.asarray(out_proj_w, np.float32))
    res = run_bass_kernel_spmd(nc, in_maps, list(range(N_CORES)))
    acc = np.zeros((H_SIZE, S), np.float64)
    for r in res.results:
        acc += np.asarray(r["outp"], np.float64).transpose(0, 2, 1, 3) \
                 .reshape(H_SIZE, S)
    return acc.T.astype(np.float32).reshape(1, S, H_SIZE)


# revision 31
# speedup vs baseline: 1.8358x; 1.0087x over previous
"""NemotronH Mamba2 mixer on 8 Trainium2 cores (Bass/Tile).

Sharding: tensor-parallel over heads/groups. Core c owns group c =
16 heads (= 1024 gate/x channels, 128 B + 128 C state channels, 16 dt).
in_proj rows and out_proj columns are sharded accordingly; out_proj is
row-parallel over the contraction, partials are combined on the host.

v2: bf16 matmul operands everywhere (fp32 kept on the decay/cumsum,
softplus and state-accumulation paths), per-head cs broadcast via
full-rate fp32r matmuls (free dim 256 over chunk pairs), softplus and
rsqrt via the exp+ln activation table (two table loads per superblock),
conv taps on GPSIMD, per-head scalars applied with stride-0 3D
broadcast APs in single wide DVE ops, bf16 output partials.
"""

import numpy as np
import ml_dtypes

import concourse.bass as bass
import concourse.mybir as mybir
from concourse import bacc
from concourse.tile import TileContext
from concourse.bass_utils import run_bass_kernel_spmd

F32 = mybir.dt.float32
F32R = mybir.dt.float32r
BF16 = mybir.dt.bfloat16
AF = mybir.ActivationFunctionType
ALU = mybir.AluOpType
NPBF16 = ml_dtypes.bfloat16

# Model dims
H_SIZE = 4096
NH = 128
HD = 64
SS = 128
KCONV = 4
NG = 8
CHUNK = 128
INTER = NH * HD                 # 8192
CONV_DIM = INTER + 2 * NG * SS  # 10240
PROJ = INTER + CONV_DIM + NH    # 18560
DT_MIN, DT_MAX = 0.001, 100.0
EPS = 1e-5
GROUP = INTER // NG             # 1024

# Sharding / tiling
N_CORES = 8
S = 2048
HL = NH // N_CORES              # 16 local heads
CLOC = HL * HD                  # 1024 local gate/x channels
NSB = 4                         # seq superblocks
SB = S // NSB                   # 512
NCPB = SB // CHUNK              # 4 chunks per superblock
NF = 19                         # f-tiles: 8 gate + 8 x + B + C + dt
NK1 = H_SIZE // 128             # 32 k-tiles for in_proj
NK2 = CLOC // 128               # 8 k-tiles for out_proj
NM2 = H_SIZE // 128             # 32 m-tiles for out_proj

_CACHE = {}


def r32(ap):
    return ap.bitcast(F32R)


def build_nc():
    nc = bacc.Bacc(None, target_bir_lowering=False)

    # hidden, pre-tiled: [sb, half, 128, 16*512] bf16 (chan-major k-tiles)
    hids = nc.declare_dram_parameter("hids", [NSB, 2, 128, 16 * SB], BF16,
                                     isOutput=False)
    # in_proj weights per f-tile: [f, half, 128, 16*128] bf16
    w1f = nc.declare_dram_parameter("w1f", [NF, 2, 128, 16 * 128], BF16,
                                    isOutput=False)
    # out_proj weights, groups of 4 m-tiles: [m/4, 128, 4*8*128] bf16
    w2m = nc.declare_dram_parameter("w2m", [NM2 // 4, 128, 4 * NK2 * 128],
                                    BF16, isOutput=False)
    convw = nc.declare_dram_parameter("convw", [128, 10 * KCONV], F32,
                                      isOutput=False)
    convb = nc.declare_dram_parameter("convb", [128, 10], F32, isOutput=False)
    dtbias = nc.declare_dram_parameter("dtbias", [HL, 1], F32, isOutput=False)
    acol = nc.declare_dram_parameter("acol", [HL, 1], F32, isOutput=False)
    dbc = nc.declare_dram_parameter("dbc", [128, HL], F32, isOutput=False)
    idf = nc.declare_dram_parameter("idf", [128, 128], F32, isOutput=False)
    idb = nc.declare_dram_parameter("idb", [128, 128], BF16, isOutput=False)
    trim = nc.declare_dram_parameter("trim", [128, 128], BF16, isOutput=False)
    outp = nc.declare_dram_parameter("outp", [NM2, NSB, 128, SB], BF16,
                                     isOutput=True)

    with TileContext(nc) as tc:
        with tc.tile_pool(name="const", bufs=1) as cp:
            idf_sb = cp.tile([128, 128], F32, tag="idf")
            idr_sb = cp.tile([128, 128], F32R, tag="idr")
            idb_sb = cp.tile([128, 128], BF16, tag="idb")
            trim_sb = cp.tile([128, 128], BF16, tag="trim")
            cw_sb = cp.tile([128, 10 * KCONV], F32, tag="cw")
            cb_sb = cp.tile([128, 10], F32, tag="cb")
            dtb_sb = cp.tile([HL, 1], F32, tag="dtb")
            a_sb = cp.tile([HL, 1], F32, tag="acol")
            dbc_sb = cp.tile([128, HL], F32, tag="dbc")
            ones16 = cp.tile([HL, 1], F32, tag="ones16")
            zcol = cp.tile([128, 1], F32, tag="zcol")
            st_sb = cp.tile([128, HL * HD], F32R, tag="state")
            stT = cp.tile([128, HL * HD], BF16, tag="stateb")
            nc.sync.dma_start(out=idf_sb[:], in_=idf[:])
            nc.sync.dma_start(out=idr_sb[:], in_=r32(idf[:]))
            nc.sync.dma_start(out=idb_sb[:], in_=idb[:])
            nc.sync.dma_start(out=trim_sb[:], in_=trim[:])
            nc.sync.dma_start(out=cw_sb[:], in_=convw[:])
            nc.sync.dma_start(out=cb_sb[:], in_=convb[:])
            nc.sync.dma_start(out=dtb_sb[:], in_=dtbias[:])
            nc.sync.dma_start(out=a_sb[:], in_=acol[:])
            nc.sync.dma_start(out=dbc_sb[:], in_=dbc[:])
            nc.vector.memset(ones16[:], 1.0)
            nc.vector.memset(zcol[:], 0.0)

            _main_phase(nc, tc, hids, w1f, w2m, outp,
                        idf_sb, idr_sb, idb_sb, trim_sb, cw_sb, cb_sb,
                        dtb_sb, a_sb, dbc_sb, ones16, zcol, st_sb, stT)

    nc.compile()
    return nc


def _main_phase(nc, tc, hids, w1f, w2m, outp,
                idf_sb, idr_sb, idb_sb, trim_sb, cw_sb, cb_sb,
                dtb_sb, a_sb, dbc_sb, ones16, zcol, st_sb, stT):
    with tc.tile_pool(name="hid", bufs=1) as hidp, \
         tc.tile_pool(name="w1", bufs=3) as w1p, \
         tc.tile_pool(name="w2", bufs=2) as w2p, \
         tc.tile_pool(name="stage", bufs=1) as sgp, \
         tc.tile_pool(name="qst", bufs=2) as qstp, \
         tc.tile_pool(name="conv32", bufs=2) as cvp, \
         tc.tile_pool(name="pair", bufs=1) as prp, \
         tc.tile_pool(name="seg", bufs=2) as segp, \
         tc.tile_pool(name="ch", bufs=2) as chp, \
         tc.tile_pool(name="ch1", bufs=1) as ch1p, \
         tc.tile_pool(name="oev", bufs=2) as oevp, \
         tc.tile_pool(name="acc", bufs=2, space="PSUM") as accp, \
         tc.tile_pool(name="psY", bufs=2, space="PSUM") as psY, \
         tc.tile_pool(name="psPB", bufs=2, space="PSUM") as psPB, \
         tc.tile_pool(name="psT", bufs=2, space="PSUM") as psT:

        # conv input staging: 10 channel tiles (8 x, 1 B, 1 C), 3 halo + SB
        ccat = sgp.tile([128, 10 * (SB + 3)], F32, tag="ccat")
        for t in range(10):
            nc.vector.memset(ccat[:, t * (SB + 3):t * (SB + 3) + 3], 0.0)

        pending_out = []

        w2cache = {}

        def load_w2(G):
            if w2cache.get("G") == G:
                return w2cache["t"]
            w2 = w2p.tile([128, 4 * NK2 * 128], BF16, tag="w2")
            nc.sync.dma_start(out=w2[:], in_=w2m[G])
            w2cache["G"] = G
            w2cache["t"] = w2
            w2cache.setdefault("loaded", set()).add(G)
            return w2

        def emit_outproj(m, qst, sbq):
            G, g = m // 4, m % 4
            if w2cache.get("G") != G:
                # use prefetched tile if the ring already holds it
                w2 = w2cache.get("pref") if w2cache.get("prefG") == G else None
                if w2 is None:
                    w2 = w2p.tile([128, 4 * NK2 * 128], BF16, tag="w2")
                    nc.sync.dma_start(out=w2[:], in_=w2m[G])
                w2cache["G"] = G
                w2cache["t"] = w2
                w2cache["pref"] = None
            w2 = w2cache["t"]
            if g == 0 and G + 1 < NM2 // 4 and w2cache.get("prefG") != G + 1:
                pw = w2p.tile([128, 4 * NK2 * 128], BF16, tag="w2",
                              name="w2pref")
                nc.sync.dma_start(out=pw[:], in_=w2m[G + 1])
                w2cache["prefG"] = G + 1
                w2cache["pref"] = pw
            acc = accp.tile([128, SB], F32, tag="acc")
            for kt in range(NK2):
                nc.tensor.matmul(
                    acc[:], w2[:, (g * NK2 + kt) * 128:(g * NK2 + kt + 1) * 128],
                    qst[:, kt * SB:kt * SB + SB],
                    start=(kt == 0), stop=(kt == NK2 - 1))
            ev = oevp.tile([128, SB], BF16, tag="oev")
            nc.scalar.copy(ev[:], acc[:])
            nc.sync.dma_start(out=outp[m, sbq], in_=ev[:])

        for sb in range(NSB):
            w1pre = []
            for khalf in range(2):
                w1t_ = w1p.tile([128, 16 * 128], BF16, tag="w1",
                                name=f"w1pre{khalf}")
                nc.sync.dma_start(out=w1t_[:], in_=w1f[18, khalf])
                w1pre.append(w1t_)
            quarters = []
            for kq in range(8):
                hid_q = hidp.tile([128, 4 * SB], BF16, tag=f"hid{kq}",
                                  name=f"hid{kq}")
                nc.sync.dma_start(
                    out=hid_q[:],
                    in_=hids[sb, kq // 4][:, (kq % 4) * 4 * SB:
                                          (kq % 4 + 1) * 4 * SB])
                quarters.append(hid_q)

            gate_sb = sgp.tile([128, 8 * SB], BF16, tag="gate")
            dtraw = sgp.tile([HL, SB], F32, tag="dtraw")

            # halo copies must read previous superblock before overwrite
            if sb > 0:
                for t in range(10):
                    base = t * (SB + 3)
                    nc.vector.tensor_copy(
                        ccat[:, base:base + 3], ccat[:, base + SB:base + SB + 3])

            def emit_ftile(f, gate_sb=gate_sb, dtraw=dtraw,
                           quarters=quarters, w1pre=w1pre):
                if f == 18:
                    w1h = w1pre
                else:
                    w1h = []
                    for khalf in range(2):
                        w1t_ = w1p.tile([128, 16 * 128], BF16, tag="w1")
                        nc.sync.dma_start(out=w1t_[:], in_=w1f[f, khalf])
                        w1h.append(w1t_)
                acc = accp.tile([128, SB], F32, tag="acc")
                for k in range(NK1):
                    nc.tensor.matmul(
                        acc[:],
                        w1h[k // 16][:, (k % 16) * 128:(k % 16 + 1) * 128],
                        quarters[k // 4][:, (k % 4) * SB:(k % 4 + 1) * SB],
                        start=(k == 0), stop=(k == NK1 - 1))
                if f < 8:
                    nc.scalar.copy(gate_sb[:, f * SB:(f + 1) * SB], acc[:])
                elif f < 18:
                    t = f - 8
                    base = t * (SB + 3)
                    nc.scalar.copy(ccat[:, base + 3:base + 3 + SB], acc[:])
                else:
                    nc.scalar.copy(dtraw[:, :], acc[:HL, :])

            def emit_dt_pipeline():
                az = sgp.tile([HL, SB], F32, tag="az")
                dtsp = dtraw  # in-place: relu(z)+ln1p overwrites raw dt
                nc.scalar.activation(az[:], dtraw[:], AF.Abs,
                                     bias=dtb_sb[:, 0:1])
                nc.scalar.activation(az[:], az[:], AF.Exp, scale=-1.0)
                nc.vector.tensor_scalar(az[:], az[:], 1.0, None, ALU.add)
                nc.scalar.activation(az[:], az[:], AF.Ln)
                nc.scalar.activation(dtsp[:], dtraw[:], AF.Relu,
                                     bias=dtb_sb[:, 0:1])
                nc.vector.tensor_tensor(dtsp[:], dtsp[:], az[:], ALU.add)
                nc.vector.tensor_scalar(dtsp[:], dtsp[:], DT_MIN, DT_MAX,
                                        ALU.max, ALU.min)
                dA = az  # az dead, reuse
                nc.vector.tensor_scalar(dA[:], dtsp[:], a_sb[:, 0:1], None,
                                        ALU.mult)
                for cl in range(NCPB):
                    ones_b = bass.AP(tensor=ones16.tensor,
                                     offset=ones16[:].offset,
                                     ap=[ones16[:].ap[0], [0, CHUNK]])
                    nc.vector.tensor_tensor_scan(
                        cs[:, cl * CHUNK:(cl + 1) * CHUNK],
                        ones_b, dA[:, cl * CHUNK:(cl + 1) * CHUNK],
                        0.0, ALU.mult, ALU.add)
                return dtsp

            cs = sgp.tile([HL, SB], F32, tag="cs")
            drain = list(pending_out)
            pending_out.clear()
            for u in [18, 16, 17] + list(range(8, 16)) + list(range(8)):
                emit_ftile(u)
                if u == 18:
                    dtsp = emit_dt_pipeline()

            # SiLU on gate (batched with conv SiLUs: one table load)
            nc.scalar.activation(gate_sb[:], gate_sb[:], AF.Silu)

            # csT/dtT for all chunks: [128, cl*HL + h] / [128, (4+cl)*HL + h]
            pcs = psPB.tile([128, 2 * NCPB * HL], F32, tag="pb",
                            name="pcs")
            for cl in range(NCPB):
                nc.tensor.transpose(
                    pcs[:, cl * HL:(cl + 1) * HL],
                    cs[:, cl * CHUNK:(cl + 1) * CHUNK], idf_sb[:HL, :HL])
                nc.tensor.transpose(
                    pcs[:, (NCPB + cl) * HL:(NCPB + cl + 1) * HL],
                    dtsp[:, cl * CHUNK:(cl + 1) * CHUNK], idf_sb[:HL, :HL])
            csdtT = sgp.tile([128, 2 * NCPB * HL], F32, tag="csdtT")
            nc.scalar.copy(csdtT[:], pcs[:])
            negcsT = sgp.tile([128, NCPB * HL], F32, tag="negcsT")
            nc.vector.tensor_scalar(negcsT[:], csdtT[:, :NCPB * HL], -1.0,
                                    None, ALU.mult)

            # bf16 triple splits of cs for both pairs (before conv: DVE order)
            splits = []
            for pr2 in range(NCPB // 2):
                p2sl = slice(pr2 * 2 * CHUNK, (pr2 + 1) * 2 * CHUNK)
                csh = sgp.tile([HL, 2 * CHUNK], BF16, tag="csh", bufs=2,
                               name=f"csh{pr2}")
                csm = sgp.tile([HL, 2 * CHUNK], BF16, tag="csm", bufs=2,
                               name=f"csm{pr2}")
                csl_ = sgp.tile([HL, 2 * CHUNK], BF16, tag="csl", bufs=2,
                               name=f"csl{pr2}")
                res = sgp.tile([HL, 2 * CHUNK], F32, tag="csres", bufs=2,
                               name=f"res{pr2}")
                nc.vector.tensor_copy(csh[:], cs[:, p2sl])
                nc.vector.tensor_tensor(res[:], cs[:, p2sl], csh[:],
                                        ALU.subtract)
                nc.vector.tensor_copy(csm[:], res[:])
                nc.vector.tensor_tensor(res[:], res[:], csm[:], ALU.subtract)
                nc.vector.tensor_copy(csl_[:], res[:])
                splits.append((csh, csm, csl_))

            # ---------------- conv taps (GPSIMD) + SiLU ----------------
            xcs = sgp.tile([128, 8 * SB], BF16, tag="xcs")
            bcs = sgp.tile([128, SB], BF16, tag="bcs")
            ccs = sgp.tile([128, SB], BF16, tag="ccs")
            for t in [8, 9] + list(range(8)):
                base = t * (SB + 3)
                eng = nc.vector
                c32 = cvp.tile([128, SB], F32, tag="c32")
                eng.tensor_scalar(
                    c32[:], ccat[:, base:base + SB],
                    cw_sb[:, t * KCONV:t * KCONV + 1], cb_sb[:, t:t + 1],
                    ALU.mult, ALU.add)
                for j in range(1, KCONV):
                    eng.scalar_tensor_tensor(
                        c32[:], ccat[:, base + j:base + j + SB],
                        cw_sb[:, t * KCONV + j:t * KCONV + j + 1], c32[:],
                        ALU.mult, ALU.add)
                dst = (xcs[:, t * SB:(t + 1) * SB] if t < 8
                       else (bcs[:] if t == 8 else ccs[:]))
                nc.scalar.activation(dst, c32[:], AF.Silu)


            qstage = qstp.tile([128, NK2 * SB], BF16, tag="qstage")
            ygat = sgp.tile([128, NCPB * CLOC], BF16, tag="ygat")
            ssum = sgp.tile([128, NCPB], F32, tag="ssum")

            # ---------------- SSD chunk pairs ----------------
            for pr in range(NCPB // 2):
                prsl = slice(pr * 2 * CHUNK, (pr + 1) * 2 * CHUNK)
                csh, csm, csl_ = splits[pr]
                # per-head cs broadcast: pb[p, j*256+l] = cs[h, pr*256+l]
                # (fp32r, free 256 -> full rate), 2 heads per PSUM tile
                epb = prp.tile([128, HL * 2 * CHUNK], F32R, tag="epb")
                segs = [segp.tile([128, HL * CHUNK], F32R, tag="seg",
                                  name=f"seg{i}")
                        for i in range(2)]
                for hg in range(HL // 2):
                    pb = psPB.tile([128, 512], F32, tag="pb")
                    for j in range(2):
                        h = 2 * hg + j
                        idcol = idb_sb[:HL, h:h + 1]
                        indh = bass.AP(tensor=idcol.tensor,
                                       offset=idcol.offset,
                                       ap=[[idcol.ap[0][0], HL], [0, 128]])
                        for si, spl in enumerate((csh, csm, csl_)):
                            nc.tensor.matmul(pb[:, j * 256:(j + 1) * 256],
                                             indh, spl[:],
                                             start=(si == 0), stop=(si == 2))
                    # seg[s, l] = min(cs[h,l] - cs[h,s], 0) per chunk
                    for lc in range(2):
                        cl = 2 * pr + lc
                        for j in range(2):
                            h = 2 * hg + j
                            nc.vector.scalar_tensor_tensor(
                                segs[lc][:, h * CHUNK:(h + 1) * CHUNK],
                                pb[:, j * 256 + lc * 128:
                                   j * 256 + (lc + 1) * 128],
                                negcsT[:, cl * HL + h:cl * HL + h + 1],
                                bass.AP(tensor=zcol.tensor,
                                        offset=zcol[:].offset,
                                        ap=[zcol[:].ap[0], [0, CHUNK]]),
                                ALU.add, ALU.min)
                    nc.scalar.activation(epb[:, hg * 512:(hg + 1) * 512],
                                         pb[:], AF.Exp)

                for lc in range(2):
                    cl = 2 * pr + lc
                    def filler(n, drain=drain):
                        for _ in range(min(n, len(drain))):
                            emit_outproj(*drain.pop(0))
                    _emit_chunk(nc, sb * NCPB + cl, cl, lc, gate_sb, xcs,
                                bcs, ccs,
                                csdtT, segs[lc], epb, ygat, ssum,
                                idf_sb, idr_sb, idb_sb, trim_sb, dbc_sb,
                                st_sb, stT, chp, ch1p, psY, psT, psPB,
                                filler)

            # ---------------- deferred group RMSNorm + transposes ----------
            lnm = chp.tile([128, NCPB], F32, tag="lnm")
            rstd = chp.tile([128, NCPB], F32, tag="rstd")
            nc.vector.tensor_scalar(lnm[:], ssum[:], 1.0 / GROUP, EPS,
                                    ALU.mult, ALU.add)
            nc.scalar.activation(lnm[:], lnm[:], AF.Ln)
            nc.scalar.activation(rstd[:], lnm[:], AF.Exp, scale=-0.5)
            for cl in range(NCPB):
                normed = ch1p.tile([128, CLOC], BF16, tag="normed")
                nc.vector.tensor_scalar(
                    normed[:], ygat[:, cl * CLOC:(cl + 1) * CLOC],
                    rstd[:, cl:cl + 1], None, ALU.mult)
                nps = psT.tile([128, CLOC], BF16, tag="trans")
                for t in range(NK2):
                    nc.tensor.transpose(
                        nps[:, t * 128:(t + 1) * 128],
                        normed[:, t * 128:(t + 1) * 128], idb_sb[:])
                qdst = qstage[:].rearrange(
                    "p (t s) -> p t s", t=NK2)[:, :, cl * 128:(cl + 1) * 128]
                nsrc = nps[:].rearrange("p (t s) -> p t s", t=NK2)
                nc.scalar.copy(qdst, nsrc)

            while drain:
                emit_outproj(*drain.pop(0))
            pending_out.extend((m, qstage, sb) for m in range(NM2))

        while pending_out:
            emit_outproj(*pending_out.pop(0))


def _emit_chunk(nc, gc, cl, lc, gate_sb, xcs, bcs, ccs,
                csdtT, seg, epb, ygat, ssum,
                idf_sb, idr_sb, idb_sb, trim_sb, dbc_sb,
                st_sb, stT, chp, ch1p, psY, psT, psPB, filler):
    csl = slice(cl * CHUNK, (cl + 1) * CHUNK)

    # gate transpose (silu already applied) -> silg_c
    gps = psT.tile([128, CLOC], BF16, tag="trans")
    for t in range(8):
        nc.tensor.transpose(
            gps[:, t * 128:(t + 1) * 128],
            gate_sb[:, t * SB + cl * CHUNK:t * SB + (cl + 1) * CHUNK],
            idb_sb[:])
    silg = chp.tile([128, CLOC], BF16, tag="silg")
    nc.scalar.copy(silg[:], gps[:])

    # scores = exp(seg) * (triu-in-[s,l] . gram); gram^T = B C^T in [s, l]
    gram_ps = psPB.tile([128, 128], F32, tag="pb", name="gram_ps")
    nc.tensor.matmul(gram_ps[:], bcs[:, csl], ccs[:, csl],
                     start=True, stop=True)
    gram = chp.tile([128, 128], F32, tag="gramm")
    nc.vector.tensor_tensor(gram[:], gram_ps[:], trim_sb[:], ALU.mult)

    # chunk-end decay per head: cend = exp(cs_end), decT = exp(cs_end - cs)
    # (both extracted BEFORE seg/epb are overwritten in place below)
    cend = chp.tile([128, HL], F32, tag="cend")
    ep1 = epb[:, (lc + 1) * CHUNK - 1:(lc + 1) * CHUNK]
    epb_end = bass.AP(tensor=epb.tensor, offset=ep1.offset,
                      ap=[ep1.ap[0], [2 * CHUNK, HL]])
    nc.vector.tensor_copy(cend[:], epb_end)
    decT = chp.tile([128, HL], F32, tag="decT")
    # seg column l=CHUNK-1 holds cs_end - cs[s] (<=0, min-clamp no-op there)
    sg1 = seg[:, CHUNK - 1:CHUNK]
    seg_end = bass.AP(tensor=seg.tensor, offset=sg1.offset,
                      ap=[sg1.ap[0], [CHUNK, HL]])
    nc.scalar.activation(decT[:], seg_end, AF.Exp)

    # scores = exp(seg) * gram -> bf16
    scores = chp.tile([128, HL * CHUNK], BF16, tag="scores", bufs=1)
    nc.scalar.activation(scores[:], seg[:], AF.Exp)
    s3 = scores[:].rearrange("p (h l) -> p h l", h=HL)
    gram_b = bass.AP(tensor=gram.tensor, offset=gram[:].offset,
                     ap=[gram[:].ap[0], [0, HL], [1, 128]])
    nc.vector.tensor_tensor(s3, s3, gram_b, ALU.mult)

    # e4 = exp(pb) * C (for Yoff) -> bf16
    e4 = chp.tile([128, HL * CHUNK], BF16, tag="e4", bufs=1)
    e4_3 = e4[:].rearrange("p (h l) -> p h l", h=HL)
    ep0 = epb[:, lc * CHUNK:lc * CHUNK + 1]
    epb_3 = bass.AP(tensor=epb.tensor, offset=ep0.offset,
                    ap=[ep0.ap[0], [2 * CHUNK, HL], [1, CHUNK]])
    cc0 = ccs[:, cl * CHUNK:cl * CHUNK + 1]
    ccs_b = bass.AP(tensor=ccs.tensor, offset=cc0.offset,
                    ap=[cc0.ap[0], [0, HL], [1, CHUNK]])
    nc.vector.tensor_tensor(e4_3, epb_3, ccs_b, ALU.mult)
    ddt = chp.tile([128, HL], F32, tag="ddt")
    nc.vector.tensor_tensor(ddt[:], csdtT[:, (NCPB + cl) * HL:
                                           (NCPB + cl + 1) * HL],
                            decT[:], ALU.mult)

    # x transpose -> xT (f32), then xdt / xdd
    xT = ch1p.tile([128, CLOC], BF16, tag="xT")
    for hx in range(2):
        xps = psT.tile([128, 512], BF16, tag="trans", name=f"xps{hx}")
        for t in range(4):
            tt = hx * 4 + t
            nc.tensor.transpose(
                xps[:, t * 128:(t + 1) * 128],
                xcs[:, tt * SB + cl * CHUNK:tt * SB + (cl + 1) * CHUNK],
                idb_sb[:])
        nc.scalar.copy(xT[:, hx * 512:(hx + 1) * 512], xps[:])
    xdt = ch1p.tile([128, CLOC], BF16, tag="xdt")
    x3 = xT[:].rearrange("p (h j) -> p h j", h=HL)
    dt0 = csdtT[:, (NCPB + cl) * HL:(NCPB + cl) * HL + 1]
    dt_b = bass.AP(tensor=csdtT.tensor, offset=dt0.offset,
                   ap=[dt0.ap[0], [1, HL], [0, HD]])
    ddt_b = bass.AP(tensor=ddt.tensor, offset=ddt[:].offset,
                    ap=[ddt[:].ap[0], [1, HL], [0, HD]])
    nc.vector.tensor_tensor(xdt[:].rearrange("p (h j) -> p h j", h=HL),
                            x3, dt_b, ALU.mult)
    # ysb = x*D now (before xdd overwrites xT in place)
    dbc_b = bass.AP(tensor=dbc_sb.tensor, offset=dbc_sb[:].offset,
                    ap=[dbc_sb[:].ap[0], [1, HL], [0, HD]])
    ysb = ch1p.tile([128, CLOC], F32, tag="ysb")
    nc.vector.tensor_tensor(ysb[:].rearrange("p (h j) -> p h j", h=HL),
                            x3, dbc_b, ALU.mult)
    xdd = xT  # in place: x * ddt overwrites xT
    nc.vector.tensor_tensor(xdd[:].rearrange("p (h j) -> p h j", h=HL),
                            x3, ddt_b, ALU.mult)

    # B chunk transposed (bln) for state matmuls
    pbt = psPB.tile([128, 128], BF16, tag="pb", name="pbt")
    nc.tensor.transpose(pbt[:], bcs[:, csl], idb_sb[:])
    bln = chp.tile([128, 128], BF16, tag="bln")
    nc.scalar.copy(bln[:], pbt[:])

    filler(3)

    # Ydiag + Yoff accumulated per head (two matmuls per head)
    y_halves = []
    for half in range(2):
        y_ps = psY.tile([128, 512], F32, tag="y", name=f"y{half}")
        for hh in range(8):
            h = half * 8 + hh
            hs = slice(hh * HD, (hh + 1) * HD)
            nc.tensor.matmul(
                y_ps[:, hs], scores[:, h * CHUNK:(h + 1) * CHUNK],
                xdt[:, h * HD:(h + 1) * HD], start=True, stop=(gc == 0))
            if gc > 0:
                nc.tensor.matmul(
                    y_ps[:, hs],
                    e4[:, h * CHUNK:(h + 1) * CHUNK],
                    stT[:, h * HD:(h + 1) * HD], start=False, stop=True)
        y_halves.append(y_ps)

    # states for this chunk
    s_halves = []
    for half in range(2):
        s_ps = psY.tile([128, 512], F32, tag="y", name=f"s{half}")
        nc.tensor.matmul(
            s_ps[:], bln[:], xdd[:, half * 512:(half + 1) * 512],
            start=True, stop=True)
        s_halves.append(s_ps)

    filler(5)

    # y = (Ydiag + Yoff) + D*x, gated; squares accumulated for RMS
    ysl = slice(cl * CLOC, (cl + 1) * CLOC)
    for half in range(2):
        hsl = slice(half * 512, (half + 1) * 512)
        nc.vector.tensor_tensor(ysb[:, hsl], ysb[:, hsl],
                                y_halves[half][:], ALU.add)
    nc.vector.tensor_tensor(ysb[:], ysb[:], silg[:], ALU.mult)
    nc.scalar.copy(ygat[:, ysl], ysb[:])
    nc.scalar.activation(xdt[:], ysb[:], AF.Square,
                         accum_out=ssum[:, cl:cl + 1])

    # state update: st = st * exp(cs_end) + s  (first chunk: st = s)
    if gc == 0:
        for half in range(2):
            hsl = slice(half * 512, (half + 1) * 512)
            nc.vector.tensor_copy(st_sb[:, hsl], s_halves[half][:])
        nc.vector.tensor_copy(stT[:], st_sb[:])
    else:
        cend_b = bass.AP(tensor=cend.tensor, offset=cend[:].offset,
                         ap=[cend[:].ap[0], [1, HL], [0, HD]])
        st3 = st_sb[:].rearrange("p (h j) -> p h j", h=HL)
        nc.vector.tensor_tensor(st3, st3, cend_b, ALU.mult)
        for half in range(2):
            hsl = slice(half * 512, (half + 1) * 512)
            nc.vector.tensor_tensor(st_sb[:, hsl], st_sb[:, hsl],
                                    s_halves[half][:], ALU.add)
        nc.vector.tensor_copy(stT[:], st_sb[:])


def prepare_in_maps(hidden_states, in_proj_w, conv_w, conv_b, dt_bias, D,
                    norm_w, out_proj_w):
    hidT = np.ascontiguousarray(hidden_states.reshape(S, H_SIZE).T)
    # [half, kk, r, sb, c] -> [sb, half, r, kk, c]
    hids = np.ascontiguousarray(
        hidT.reshape(2, 16, 128, NSB, SB).transpose(3, 0, 2, 1, 4)
        .reshape(NSB, 2, 128, 16 * SB)).astype(NPBF16)
    idf = np.eye(128, dtype=np.float32)
    idb = np.eye(128).astype(NPBF16)
    # mask in [s, l]: keep l >= s
    trim = np.triu(np.ones((128, 128), np.float32)).astype(NPBF16)
    in_maps = []
    for c in range(N_CORES):
        gsl = slice(CLOC * c, CLOC * (c + 1))
        xsl = slice(INTER + CLOC * c, INTER + CLOC * (c + 1))
        bsl = slice(2 * INTER + SS * c, 2 * INTER + SS * (c + 1))
        cslc = slice(2 * INTER + NG * SS + SS * c,
                     2 * INTER + NG * SS + SS * (c + 1))
        dsl = slice(INTER + CONV_DIM + HL * c, INTER + CONV_DIM + HL * (c + 1))
        w1 = np.concatenate([in_proj_w[gsl], in_proj_w[xsl], in_proj_w[bsl],
                             in_proj_w[cslc], in_proj_w[dsl]], axis=0)
        w1 = np.concatenate(
            [w1, np.zeros((NF * 128 - w1.shape[0], H_SIZE), np.float32)],
            axis=0)
        w1f = np.ascontiguousarray(
            w1.T.reshape(2, 16, 128, NF, 128).transpose(3, 0, 2, 1, 4)
            .reshape(NF, 2, 128, 16 * 128)).astype(NPBF16)
        w2 = out_proj_w[:, gsl] * norm_w[gsl][None, :]  # norm_w folded
        w2m = np.ascontiguousarray(
            w2.T.reshape(NK2, 128, NM2, 128).transpose(2, 1, 0, 3)
            .reshape(NM2 // 4, 4 * 128 * NK2 * 128 // 1024, 1024)
            .reshape(NM2 // 4, 4, 128, NK2 * 128).transpose(0, 2, 1, 3)
            .reshape(NM2 // 4, 128, 4 * NK2 * 128)).astype(NPBF16)
        conv_idx = np.concatenate([
            np.arange(CLOC * c, CLOC * (c + 1)),
            np.arange(INTER + SS * c, INTER + SS * (c + 1)),
            np.arange(INTER + NG * SS + SS * c,
                      INTER + NG * SS + SS * (c + 1))])
        cwl = conv_w[conv_idx, 0, :]          # [1280, 4]
        cbl = conv_b[conv_idx]                # [1280]
        convw = np.ascontiguousarray(
            cwl.reshape(10, 128, KCONV).transpose(1, 0, 2)
            .reshape(128, 10 * KCONV))
        convb = np.ascontiguousarray(cbl.reshape(10, 128).transpose(1, 0))
        hsl = slice(HL * c, HL * (c + 1))
        acol = -(np.arange(HL * c + 1, HL * (c + 1) + 1, dtype=np.float32))
        in_maps.append({
            "hids": hids,
            "w1f": w1f,
            "w2m": w2m,
            "convw": convw,
            "convb": convb,
            "dtbias": dt_bias[hsl].reshape(HL, 1).astype(np.float32),
            "acol": acol.reshape(HL, 1),
            "dbc": np.tile(D[hsl][None, :], (128, 1)).astype(np.float32),
            "idf": idf,
            "idb": idb,
            "trim": trim,
        })
    return in_maps


def get_nc():
    if "nc" not in _CACHE:
        _CACHE["nc"] = build_nc()
    return _CACHE["nc"]


def kernel(hidden_states, in_proj_w, conv_w, conv_b, dt_bias, D, norm_w,
           out_proj_w):
    nc = get_nc()
    in_maps = prepare_in_maps(
        np.asarray(hidden_states, np.float32),
        np.asarray(in_proj_w, np.float32),
        np.asarray(conv_w, np.float32), np.asarray(conv_b, np.float32),
        np.asarray(dt_bias, np.float32), np.asarray(D, np.float32),
        np.asarray(norm_w, np.float32), np.asarray(out_proj_w, np.float32))
    res = run_bass_kernel_spmd(nc, in_maps, list(range(N_CORES)))
    acc = np.zeros((H_SIZE, S), np.float64)
    for r in res.results:
        acc += np.asarray(r["outp"], np.float64).transpose(0, 2, 1, 3) \
                 .reshape(H_SIZE, S)
    return acc.T.astype(np.float32).reshape(1, S, H_SIZE)
